# revision 1
# baseline (speedup 1.0000x reference)
"""Causal MHA (shared q_linear) Bass kernel for 8 TRN2 NeuronCores.

Sharding: core c handles batch b=c//2, head-group g=c%2 (8 of 16 heads,
columns 512g:512g+512 of the shared projection).  Each core computes a
partial output (its head-group's contribution through Wo); the host sums
the two partials per batch and adds bo.

Compute layout (per core, S=2048 tokens, D=1024, 8 heads of hd=64):
  xT  = transpose(x) via PE               [1024, 2048]  (fp32, exact)
  qT/kT = Wq_g^T @ xT (+bq)               [512, 2048]   (fp32r matmuls)
  v   = x @ Wq_g (+bq), stored [tok, head, 65] with a fused ones column
  scoresT[k,q] = kh @ qh^T (per head, K=64, two heads packed in PE rows)
  exp on ACT with scale=1/8, additive -1e10 causal mask on PSUM
  attnT[hd+1, q] = [vh|1]^T @ expT  accumulated over k in PSUM
     row 64 = sum(exp) -> reciprocal -> rank-1 PE broadcast -> normalize
  out = attnT^T @ Wo_g  (partial, host adds the two head-groups + bo)
"""

import sys

sys.path.insert(0, "/opt/trn_rl_repo")

import numpy as np
import concourse.bass as bass  # noqa: F401
import concourse.tile as tile
from concourse import bacc, mybir
from concourse.bass_utils import run_bass_kernel_spmd

F32 = mybir.dt.float32
F32R = mybir.dt.float32r
BF16 = mybir.dt.bfloat16
AF = mybir.ActivationFunctionType

S = 2048          # tokens
D = 1024          # model dim
DL = 512          # local (per-core) projection columns = 8 heads * 64
HD = 64           # head dim
NHL = 8           # local heads
TB = 4            # token blocks of 512
JD = 8            # Din blocks of 128
NEG = -1.0e10


def build(repeat: int = 1, mode: str = "full", variant: str = "v4"):
    nc = bacc.Bacc("TRN2", target_bir_lowering=False, debug=False)
    xdt = BF16 if variant == "v7" else F32
    x_aps = {
        n: nc.dram_tensor(n, [S, D], xdt, kind="ExternalInput").ap()
        for n in ("x_q", "x_k", "x_v")
    }
    wq_ap = nc.dram_tensor("wq", [D, DL], F32, kind="ExternalInput").ap()
    bq_ap = nc.dram_tensor("bq", [DL], F32, kind="ExternalInput").ap()
    wo_ap = nc.dram_tensor("wo", [DL, D], F32, kind="ExternalInput").ap()
    tri_ap = nc.dram_tensor("tri", [128, 128], F32, kind="ExternalInput").ap()
    tri01_ap = nc.dram_tensor("tri01", [128, 128], F32, kind="ExternalInput").ap()
    id_ap = nc.dram_tensor("ident", [128, 128], F32, kind="ExternalInput").ap()
    out_ap = nc.dram_tensor("out", [S, D], F32, kind="ExternalOutput").ap()

    with tile.TileContext(nc) as tc:
        with tc.tile_pool(name="const", bufs=1) as const, \
             tc.tile_pool(name="persist", bufs=1) as persist, \
                          tc.tile_pool(name="xn", bufs=3) as xnp, \
             tc.tile_pool(name="xt", bufs=1) as xtp, \
             tc.tile_pool(name="qt", bufs=4) as qtp, \
             tc.tile_pool(name="exp", bufs=(2 if variant == "v9" else 3)) as ep, \
             tc.tile_pool(name="at", bufs=2) as atp, \
             tc.tile_pool(name="norm", bufs=1) as normp, \
             tc.tile_pool(name="ob", bufs=2) as obp, \
             tc.tile_pool(name="psS", bufs=(2 if variant == "v8" else 3), space="PSUM") as psS, \
             tc.tile_pool(name="psAcc", bufs=(3 if variant == "v8" else 2), space="PSUM") as psAcc:

            # ---- constants ----
            ident = const.tile([128, 128], F32)
            nc.sync.dma_start(ident[:], id_ap[:])
            tri = const.tile([128, 128], F32)
            nc.sync.dma_start(tri[:], tri_ap[:])
            tri01 = const.tile([128, 128], F32)
            nc.sync.dma_start(tri01[:], tri01_ap[:])
            bq_sb = const.tile([128, 4], F32)
            nc.sync.dma_start(bq_sb[:], bq_ap.rearrange("(t p) -> p t", p=128))
            bq_row = const.tile([1, DL], F32)
            nc.sync.dma_start(bq_row[:], bq_ap.rearrange("(a n) -> a n", a=1))
            bq_row_r = const.tile([1, DL], F32R)
            nc.vector.tensor_copy(bq_row_r[:], bq_row[:])
            ones_f = const.tile([128, 128], F32)
            nc.vector.memset(ones_f[:], 1.0)
            ones_r = const.tile([128, 128], F32R)
            nc.vector.tensor_copy(ones_r[:], ones_f[:])
            if variant == "v7":
                ones_b = const.tile([128, 128], BF16)
                nc.vector.tensor_copy(ones_b[:], ones_f[:])
                bq_row_b = const.tile([1, DL], BF16)
                nc.vector.tensor_copy(bq_row_b[:], bq_row[:])

            # ---- weights (cast to fp32r once) ----
            wq_r = persist.tile([128, JD, DL], BF16 if variant == "v7" else F32R)
            for j in range(JD):
                st = obp.tile([128, D], F32, tag="ob")
                nc.sync.dma_start(st[:, 0:DL], wq_ap[j * 128:(j + 1) * 128, :])
                nc.vector.tensor_copy(wq_r[:, j, :], st[:, 0:DL])
            wo_r = persist.tile([128, 4, D], F32R)
            for kt in range(4):
                st = obp.tile([128, D], F32, tag="ob")
                nc.sync.dma_start(st[:], wo_ap[kt * 128:(kt + 1) * 128, :])
                nc.vector.tensor_copy(wo_r[:, kt, :], st[:])

            # persistent per-token-block tensors
            kT = [persist.tile([128, 4, 512], F32R, name=f"kT{i}", tag=f"kT{i}") for i in range(TB)]
            vv = [persist.tile([128, 4, NHL, HD + 1], F32R, name=f"vv{i}", tag=f"vv{i}") for i in range(TB)]

            q_tiles = [None] * TB

            def _phase1_transpose(x_ap, xT, tb):
                for sub in range(4):
                    r0 = tb * 512 + sub * 128
                    # two half-tiles so transposes of D-cols 0:512 start as
                    # soon as the first 256KB lands (whole-tile dep otherwise
                    # stalls PE ~2.7us at every input boundary)
                    xh = []
                    for half in range(2):
                        xn = xnp.tile([128, DL], F32, tag=f"xn{half}")
                        nc.sync.dma_start(
                            xn[:], x_ap[r0:r0 + 128, half * DL:(half + 1) * DL]
                        )
                        xh.append(xn)
                    if variant not in ("v5", "v6"):
                        for jg in range(2):
                            pt = psS.tile([128, 512], F32, tag="sc")
                            for ji in range(4):
                                j = jg * 4 + ji
                                nc.tensor.transpose(
                                    pt[:, ji * 128:(ji + 1) * 128],
                                    xh[jg][:, ji * 128:(ji + 1) * 128],
                                    ident[:],
                                )
                            dst = xT[:, jg * 4:(jg + 1) * 4,
                                     sub * 128:(sub + 1) * 128]
                            srcv = pt[:].rearrange("p (j t) -> p j t", j=4)
                            if jg == 0 or variant == "v3":
                                nc.vector.tensor_copy(dst, srcv)
                            else:
                                nc.scalar.activation(dst, srcv, AF.Identity)
                    else:
                        pt = psS.tile([128, 2, 512], F32, tag="sc")
                        for j in range(JD):
                            nc.tensor.transpose(
                                pt[:, j // 4, (j % 4) * 128:(j % 4 + 1) * 128],
                                xn[:, j * 128:(j + 1) * 128],
                                ident[:],
                            )
                        dst = xT[:, :, sub * 128:(sub + 1) * 128]
                        srcv = pt[:].rearrange("p b (g t) -> p (b g) t", g=4)
                        if sub % 2 == 0:
                            nc.vector.tensor_copy(dst, srcv)
                        else:
                            nc.scalar.activation(dst, srcv, AF.Identity)

            def phase1(tb, rep):
                """transpose + project q,k,v for token block tb (512 tokens)."""
                for name in ("x_k", "x_v", "x_q"):
                    x_ap = x_aps[name]
                    if variant == "v7":
                        xT = xtp.tile([128, JD, 512], BF16, tag="xt")
                        for j in range(JD):
                            nc.scalar.dma_start(
                                out=xT[:, j, :],
                                in_=x_ap[tb * 512:(tb + 1) * 512,
                                         j * 128:(j + 1) * 128],
                                transpose=True,
                            )
                    else:
                        xT = xtp.tile([128, JD, 512], F32R, tag="xt")
                        _phase1_transpose(x_ap, xT, tb)
                    if name == "x_v":
                        vt = vv[tb]
                        for sub in range(4):
                            pv = psS.tile([128, 512], F32, tag="sc")
                            for j in range(JD):
                                nc.tensor.matmul(
                                    pv[:],
                                    xT[:, j, sub * 128:(sub + 1) * 128],
                                    wq_r[:, j, :],
                                    start=(j == 0),
                                    stop=False,
                                )
                            nc.tensor.matmul(
                                pv[:],
                                (ones_b if variant == "v7" else ones_r)[0:1, 0:128],
                                (bq_row_b if variant == "v7" else bq_row_r)[:],
                                start=False,
                                stop=True,
                            )
                            nc.vector.tensor_copy(
                                vt[:, sub, :, 0:HD],
                                pv[:].rearrange("p (h d) -> p h d", h=NHL),
                            )
                        nc.vector.tensor_copy(
                            vt[:, :, :, HD],
                            ones_f[:, 0:32].rearrange("p (s h) -> p s h", s=4),
                        )
                    else:
                        if name == "x_q":
                            dest = qtp.tile([128, 4, 512], F32R, tag="qt")
                            q_tiles[tb] = dest
                        else:
                            dest = kT[tb]
                        for dt_ in range(4):
                            py = psS.tile([128, 512], F32, tag="sc")
                            for j in range(JD):
                                nc.tensor.matmul(
                                    py[:],
                                    wq_r[:, j, dt_ * 128:(dt_ + 1) * 128],
                                    xT[:, j, :],
                                    start=(j == 0),
                                    stop=(j == JD - 1),
                                )
                            nc.scalar.activation(
                                dest[:, dt_, :],
                                py[:],
                                AF.Identity,
                                bias=bq_sb[:, dt_:dt_ + 1],
                            )

            def attention(Q, rep):
                """attention + Wo for query block Q (512 tokens)."""
                attnT = [atp.tile([128, 512], F32R, tag=f"at{i}", name=f"attnT{i}")
                         for i in range(4)]
                qtile = q_tiles[Q]
                nj = 4 * (Q + 1)
                for hp in range(4):
                    acc0 = psAcc.tile([128, 512], F32, tag="acc")
                    acc1 = psAcc.tile([128, 512], F32, tag="acc")

                    def emit_scores(j):
                        """scoresT pair + mask + exp for k-tile j; returns exp tile."""
                        tbj, sub = j // 4, j % 4
                        qoff = max(0, j * 128 - Q * 512)
                        ps = psS.tile([128, 2, 512], F32, tag="sc", name=f"ps{j}")
                        for hi, base in ((0, 0), (1, 64)):
                            nc.tensor.matmul(
                                ps[:, hi, qoff:],
                                kT[tbj][base:base + 64, hp,
                                        sub * 128:(sub + 1) * 128],
                                qtile[base:base + 64, hp, qoff:],
                                start=True,
                                stop=True,
                            )
                        diag = j * 128 >= Q * 512
                        if diag and variant != "v11":
                            for hi in range(2):
                                nc.vector.tensor_add(
                                    ps[:, hi, qoff:qoff + 128],
                                    ps[:, hi, qoff:qoff + 128],
                                    tri[:],
                                )
                        et = ep.tile([128, 2, 512], F32R, tag="exp", name=f"et{j}")
                        nc.scalar.activation(
                            et[:, :, qoff:], ps[:, :, qoff:], AF.Exp, scale=0.125
                        )
                        if diag and variant == "v11":
                            # zero masked entries after exp, off the PE->ACT chain
                            for hi in range(2):
                                nc.vector.tensor_mul(
                                    et[:, hi, qoff:qoff + 128],
                                    et[:, hi, qoff:qoff + 128],
                                    tri01[:],
                                )
                        return et

                    def emit_attn(j, et):
                        tbj, sub = j // 4, j % 4
                        qoff = max(0, j * 128 - Q * 512)
                        for hi, acc in ((0, acc0), (1, acc1)):
                            nc.tensor.matmul(
                                acc[0:65, qoff:],
                                vv[tbj][:, sub, hp * 2 + hi, :],
                                et[:, hi, qoff:],
                                start=(j == 0),
                                stop=(j == nj - 1),
                            )

                    # software pipeline: scores/exp run up to two k-tiles
                    # ahead of the accumulating attn matmuls so the in-order
                    # PE stream never head-blocks on the ACT exp.
                    depth = {"v3": 1, "v6": 3}.get(variant, 2)
                    ets = [emit_scores(j) for j in range(min(depth, nj))]
                    for j in range(depth, nj):
                        ets.append(emit_scores(j))
                        emit_attn(j - depth, ets[j - depth])
                    for j in range(max(0, nj - depth), nj):
                        emit_attn(j, ets[j])
                    if variant in ("v9",):
                        accs_sb = []
                        for hi, acc in ((0, acc0), (1, acc1)):
                            asb = normp.tile([128, 512], F32, tag=f"asb{hi}")
                            nc.vector.tensor_copy(asb[0:65, :], acc[0:65, :])
                            accs_sb.append(asb)
                        for hi, asb in ((0, accs_sb[0]), (1, accs_sb[1])):
                            sr = normp.tile([1, 512], F32, tag="sr")
                            nc.vector.tensor_copy(sr[0:1, :], asb[64:65, :])
                            bb = normp.tile([64, 512], F32, tag="bb")
                            nc.gpsimd.partition_broadcast(bb[:], sr[0:1, :])
                            rb = normp.tile([64, 512], F32, tag="rb")
                            nc.vector.reciprocal(rb[:], bb[:])
                            nc.vector.tensor_mul(
                                attnT[hp][hi * 64:(hi + 1) * 64, :],
                                asb[0:64, :],
                                rb[:],
                            )
                        continue_norm = False
                    else:
                        continue_norm = True
                    for hi, acc in (((0, acc0), (1, acc1)) if continue_norm else ()):
                        if variant == "v3":
                            sr = normp.tile([128, 512], F32R, tag="srr")
                            nc.vector.tensor_copy(sr[64:65, :], acc[64:65, :])
                            pb = psS.tile([128, 512], F32, tag="sc")
                            nc.tensor.matmul(
                                pb[0:64, :], ones_r[64:65, 0:64], sr[64:65, :],
                                start=True, stop=True,
                            )
                            rb = normp.tile([64, 512], F32, tag="rb")
                            nc.vector.reciprocal(rb[:], pb[0:64, :])
                        else:
                            # sum row -> DMA partition-broadcast -> wide
                            # reciprocal -> normalize (no PE/ACT involvement)
                            sr = normp.tile([1, 512], F32, tag="sr")
                            nc.vector.tensor_copy(sr[0:1, :], acc[64:65, :])
                            bb = normp.tile([64, 512], F32, tag="bb")
                            nc.gpsimd.partition_broadcast(bb[:], sr[0:1, :])
                            rb = normp.tile([64, 512], F32, tag="rb")
                            nc.vector.reciprocal(rb[:], bb[:])
                        nc.vector.tensor_mul(
                            attnT[hp][hi * 64:(hi + 1) * 64, :],
                            acc[0:64, :],
                            rb[:],
                        )
                # Wo projection for this token block
                for st_ in range(4):
                    ob = obp.tile([128, D], F32, tag="ob")
                    for nh in range(2):
                        po = psS.tile([128, 512], F32, tag="sc")
                        for kt in range(4):
                            nc.tensor.matmul(
                                po[:],
                                attnT[kt][:, st_ * 128:(st_ + 1) * 128],
                                wo_r[:, kt, nh * 512:(nh + 1) * 512],
                                start=(kt == 0),
                                stop=(kt == 3),
                            )
                        nc.vector.tensor_copy(ob[:, nh * 512:(nh + 1) * 512], po[:])
                    r0 = Q * 512 + st_ * 128
                    nc.sync.dma_start(out_ap[r0:r0 + 128, :], ob[:])

            if mode == "full":
                for rep in range(repeat):
                    if variant == "v10":
                        phase1(0, rep)
                        phase1(1, rep)
                        attention(0, rep)
                        phase1(2, rep)
                        attention(1, rep)
                        phase1(3, rep)
                        attention(2, rep)
                        attention(3, rep)
                    else:
                        for tb in range(TB):
                            phase1(tb, rep)
                        for Q in range(TB):
                            if Q == 0 and variant == "v12":
                                with tc.high_priority():
                                    attention(Q, rep)
                            else:
                                attention(Q, rep)
            elif mode == "p1":
                for rep in range(repeat):
                    for tb in range(TB):
                        phase1(tb, rep)
                for Q in range(TB):
                    attention(Q, 0)
            elif mode == "attn":
                for tb in range(TB):
                    phase1(tb, 0)
                for rep in range(repeat):
                    for Q in range(TB):
                        attention(Q, rep)

    nc.compile()
    return nc


_BUILD_CACHE = {}


def _get(repeat=1, mode="full", variant="v4"):
    key = (repeat, mode, variant)
    if key not in _BUILD_CACHE:
        _BUILD_CACHE[key] = build(repeat, mode, variant)
    return _BUILD_CACHE[key]


def make_in_maps(q, k, v, Wq, bq, Wo, bo, variant="v4"):
    import ml_dtypes
    xdt = ml_dtypes.bfloat16 if variant == "v7" else np.float32
    tri = np.where(
        np.arange(128)[:, None] <= np.arange(128)[None, :], 0.0, NEG
    ).astype(np.float32)
    tri01 = (tri == 0.0).astype(np.float32)
    ident = np.eye(128, dtype=np.float32)
    in_maps = []
    for c in range(8):
        b, g = c // 2, c % 2
        sl = slice(g * DL, (g + 1) * DL)
        in_maps.append({
            "x_q": np.ascontiguousarray(q[b]).astype(xdt),
            "x_k": np.ascontiguousarray(k[b]).astype(xdt),
            "x_v": np.ascontiguousarray(v[b]).astype(xdt),
            "wq": np.ascontiguousarray(Wq[:, sl]),
            "bq": np.ascontiguousarray(bq[sl]),
            "wo": np.ascontiguousarray(Wo[sl, :]),
            "tri": tri,
            "tri01": tri01,
            "ident": ident,
        })
    return in_maps


DEFAULT_VARIANT = "v4"


def kernel(q, k, v, Wq, bq, Wo, bo):
    q, k, v, Wq, bq, Wo, bo = (
        np.asarray(a, dtype=np.float32) for a in (q, k, v, Wq, bq, Wo, bo)
    )
    nc = _get(1, "full", DEFAULT_VARIANT)
    in_maps = make_in_maps(q, k, v, Wq, bq, Wo, bo, DEFAULT_VARIANT)
    res = run_bass_kernel_spmd(nc, in_maps, list(range(8)))
    B = q.shape[0]
    out = np.empty((B, S, D), dtype=np.float32)
    for b in range(B):
        out[b] = res.results[2 * b]["out"] + res.results[2 * b + 1]["out"] + bo
    return out



# revision 6
# speedup vs baseline: 1.6241x; 1.6241x over previous
"""Causal MHA (shared q_linear) Bass kernel for 8 TRN2 NeuronCores.

Sharding: core c handles batch b=c//2, head-group g=c%2 (8 of 16 heads,
columns 512g:512g+512 of the shared projection).  Each core computes a
partial output (its head-group's contribution through Wo); the host sums
the two partials per batch and adds bo.

Compute layout (per core, S=2048 tokens, D=1024, 8 heads of hd=64):
  xT  = transpose(x) via PE               [1024, 2048]  (fp32, exact)
  qT/kT = Wq_g^T @ xT (+bq)               [512, 2048]   (fp32r matmuls)
  v   = x @ Wq_g (+bq), stored [tok, head, 65] with a fused ones column
  scoresT[k,q] = kh @ qh^T (per head, K=64, two heads packed in PE rows)
  exp on ACT with scale=1/8, additive -1e10 causal mask on PSUM
  attnT[hd+1, q] = [vh|1]^T @ expT  accumulated over k in PSUM
     row 64 = sum(exp) -> reciprocal -> rank-1 PE broadcast -> normalize
  out = attnT^T @ Wo_g  (partial, host adds the two head-groups + bo)
"""

import sys

sys.path.insert(0, "/opt/trn_rl_repo")

import numpy as np
import concourse.bass as bass  # noqa: F401
import concourse.tile as tile
from concourse import bacc, mybir
from concourse.bass_utils import run_bass_kernel_spmd

F32 = mybir.dt.float32
F32R = mybir.dt.float32r
BF16 = mybir.dt.bfloat16
AF = mybir.ActivationFunctionType

S = 2048          # tokens
D = 1024          # model dim
DL = 512          # local (per-core) projection columns = 8 heads * 64
HD = 64           # head dim
NHL = 8           # local heads
TB = 4            # token blocks of 512
JD = 8            # Din blocks of 128
NEG = -1.0e10


def build(repeat: int = 1, mode: str = "full", variant: str = "v4"):
    nc = bacc.Bacc("TRN2", target_bir_lowering=False, debug=False)
    xdt = BF16 if variant == "v7" else F32
    x_aps = {
        n: nc.dram_tensor(n, [S, D], xdt, kind="ExternalInput").ap()
        for n in ("x_q", "x_k", "x_v")
    }
    wq_ap = nc.dram_tensor("wq", [D, DL], F32, kind="ExternalInput").ap()
    bq_ap = nc.dram_tensor("bq", [DL], F32, kind="ExternalInput").ap()
    wo_ap = nc.dram_tensor("wo", [DL, D], F32, kind="ExternalInput").ap()
    tri_ap = nc.dram_tensor("tri", [128, 128], F32, kind="ExternalInput").ap()
    tri01_ap = nc.dram_tensor("tri01", [128, 128], F32, kind="ExternalInput").ap()
    id_ap = nc.dram_tensor("ident", [128, 128], F32, kind="ExternalInput").ap()
    out_ap = nc.dram_tensor("out", [S, D], F32, kind="ExternalOutput").ap()

    with tile.TileContext(nc) as tc:
        with tc.tile_pool(name="const", bufs=1) as const, \
             tc.tile_pool(name="persist", bufs=1) as persist, \
                          tc.tile_pool(name="xn", bufs=3) as xnp, \
             tc.tile_pool(name="xt", bufs=1) as xtp, \
             tc.tile_pool(name="qt", bufs=4) as qtp, \
             tc.tile_pool(name="exp", bufs=(2 if variant == "v9" else 3)) as ep, \
             tc.tile_pool(name="at", bufs=2) as atp, \
             tc.tile_pool(name="norm", bufs=1) as normp, \
             tc.tile_pool(name="ob", bufs=2) as obp, \
             tc.tile_pool(name="psS", bufs=(2 if variant == "v8" else 3), space="PSUM") as psS, \
             tc.tile_pool(name="psAcc", bufs=(3 if variant == "v8" else 2), space="PSUM") as psAcc:

            # ---- constants ----
            ident = const.tile([128, 128], F32)
            nc.sync.dma_start(ident[:], id_ap[:])
            tri = const.tile([128, 128], F32)
            nc.sync.dma_start(tri[:], tri_ap[:])
            tri01 = const.tile([128, 128], F32)
            nc.sync.dma_start(tri01[:], tri01_ap[:])
            bq_sb = const.tile([128, 4], F32)
            nc.sync.dma_start(bq_sb[:], bq_ap.rearrange("(t p) -> p t", p=128))
            bq_row = const.tile([1, DL], F32)
            nc.sync.dma_start(bq_row[:], bq_ap.rearrange("(a n) -> a n", a=1))
            bq_row_r = const.tile([1, DL], F32R)
            nc.vector.tensor_copy(bq_row_r[:], bq_row[:])
            ones_f = const.tile([128, 128], F32)
            nc.vector.memset(ones_f[:], 1.0)
            ones_r = const.tile([128, 128], F32R)
            nc.vector.tensor_copy(ones_r[:], ones_f[:])
            if variant == "v7":
                ones_b = const.tile([128, 128], BF16)
                nc.vector.tensor_copy(ones_b[:], ones_f[:])
                bq_row_b = const.tile([1, DL], BF16)
                nc.vector.tensor_copy(bq_row_b[:], bq_row[:])

            # ---- weights (cast to fp32r once) ----
            wq_r = persist.tile([128, JD, DL], BF16 if variant == "v7" else F32R)
            for j in range(JD):
                st = obp.tile([128, D], F32, tag="ob")
                nc.sync.dma_start(st[:, 0:DL], wq_ap[j * 128:(j + 1) * 128, :])
                nc.vector.tensor_copy(wq_r[:, j, :], st[:, 0:DL])
            wo_r = persist.tile([128, 4, D], F32R)
            for kt in range(4):
                st = obp.tile([128, D], F32, tag="ob")
                nc.sync.dma_start(st[:], wo_ap[kt * 128:(kt + 1) * 128, :])
                nc.vector.tensor_copy(wo_r[:, kt, :], st[:])

            # persistent per-token-block tensors
            kT = [persist.tile([128, 4, 512], F32R, name=f"kT{i}", tag=f"kT{i}") for i in range(TB)]
            vv = [persist.tile([128, 4, NHL, HD + 1], F32R, name=f"vv{i}", tag=f"vv{i}") for i in range(TB)]

            q_tiles = [None] * TB

            def _phase1_transpose(x_ap, xT, tb):
                for sub in range(4):
                    r0 = tb * 512 + sub * 128
                    # two half-tiles so transposes of D-cols 0:512 start as
                    # soon as the first 256KB lands (whole-tile dep otherwise
                    # stalls PE ~2.7us at every input boundary)
                    xh = []
                    for half in range(2):
                        xn = xnp.tile([128, DL], F32, tag=f"xn{half}")
                        nc.sync.dma_start(
                            xn[:], x_ap[r0:r0 + 128, half * DL:(half + 1) * DL]
                        )
                        xh.append(xn)
                    if variant not in ("v5", "v6"):
                        for jg in range(2):
                            pt = psS.tile([128, 512], F32, tag="sc")
                            for ji in range(4):
                                j = jg * 4 + ji
                                nc.tensor.transpose(
                                    pt[:, ji * 128:(ji + 1) * 128],
                                    xh[jg][:, ji * 128:(ji + 1) * 128],
                                    ident[:],
                                )
                            dst = xT[:, jg * 4:(jg + 1) * 4,
                                     sub * 128:(sub + 1) * 128]
                            srcv = pt[:].rearrange("p (j t) -> p j t", j=4)
                            if jg == 0 or variant == "v3":
                                nc.vector.tensor_copy(dst, srcv)
                            else:
                                nc.scalar.activation(dst, srcv, AF.Identity)
                    else:
                        pt = psS.tile([128, 2, 512], F32, tag="sc")
                        for j in range(JD):
                            nc.tensor.transpose(
                                pt[:, j // 4, (j % 4) * 128:(j % 4 + 1) * 128],
                                xn[:, j * 128:(j + 1) * 128],
                                ident[:],
                            )
                        dst = xT[:, :, sub * 128:(sub + 1) * 128]
                        srcv = pt[:].rearrange("p b (g t) -> p (b g) t", g=4)
                        if sub % 2 == 0:
                            nc.vector.tensor_copy(dst, srcv)
                        else:
                            nc.scalar.activation(dst, srcv, AF.Identity)

            def phase1(tb, rep):
                """transpose + project q,k,v for token block tb (512 tokens)."""
                for name in ("x_k", "x_v", "x_q"):
                    x_ap = x_aps[name]
                    if variant == "v7":
                        xT = xtp.tile([128, JD, 512], BF16, tag="xt")
                        for j in range(JD):
                            nc.scalar.dma_start(
                                out=xT[:, j, :],
                                in_=x_ap[tb * 512:(tb + 1) * 512,
                                         j * 128:(j + 1) * 128],
                                transpose=True,
                            )
                    else:
                        xT = xtp.tile([128, JD, 512], F32R, tag="xt")
                        _phase1_transpose(x_ap, xT, tb)
                    if name == "x_v":
                        vt = vv[tb]
                        for sub in range(4):
                            pv = psS.tile([128, 512], F32, tag="sc")
                            for j in range(JD):
                                nc.tensor.matmul(
                                    pv[:],
                                    xT[:, j, sub * 128:(sub + 1) * 128],
                                    wq_r[:, j, :],
                                    start=(j == 0),
                                    stop=False,
                                )
                            nc.tensor.matmul(
                                pv[:],
                                (ones_b if variant == "v7" else ones_r)[0:1, 0:128],
                                (bq_row_b if variant == "v7" else bq_row_r)[:],
                                start=False,
                                stop=True,
                            )
                            nc.vector.tensor_copy(
                                vt[:, sub, :, 0:HD],
                                pv[:].rearrange("p (h d) -> p h d", h=NHL),
                            )
                        nc.vector.tensor_copy(
                            vt[:, :, :, HD],
                            ones_f[:, 0:32].rearrange("p (s h) -> p s h", s=4),
                        )
                    else:
                        if name == "x_q":
                            dest = qtp.tile([128, 4, 512], F32R, tag="qt")
                            q_tiles[tb] = dest
                        else:
                            dest = kT[tb]
                        for dt_ in range(4):
                            py = psS.tile([128, 512], F32, tag="sc")
                            for j in range(JD):
                                nc.tensor.matmul(
                                    py[:],
                                    wq_r[:, j, dt_ * 128:(dt_ + 1) * 128],
                                    xT[:, j, :],
                                    start=(j == 0),
                                    stop=(j == JD - 1),
                                )
                            nc.scalar.activation(
                                dest[:, dt_, :],
                                py[:],
                                AF.Identity,
                                bias=bq_sb[:, dt_:dt_ + 1],
                            )

            def attention(Q, rep):
                """attention + Wo for query block Q (512 tokens)."""
                attnT = [atp.tile([128, 512], F32R, tag=f"at{i}", name=f"attnT{i}")
                         for i in range(4)]
                qtile = q_tiles[Q]
                nj = 4 * (Q + 1)
                for hp in range(4):
                    acc0 = psAcc.tile([128, 512], F32, tag="acc")
                    acc1 = psAcc.tile([128, 512], F32, tag="acc")

                    def emit_scores(j):
                        """scoresT pair + mask + exp for k-tile j; returns exp tile."""
                        tbj, sub = j // 4, j % 4
                        qoff = max(0, j * 128 - Q * 512)
                        ps = psS.tile([128, 2, 512], F32, tag="sc", name=f"ps{j}")
                        for hi, base in ((0, 0), (1, 64)):
                            nc.tensor.matmul(
                                ps[:, hi, qoff:],
                                kT[tbj][base:base + 64, hp,
                                        sub * 128:(sub + 1) * 128],
                                qtile[base:base + 64, hp, qoff:],
                                start=True,
                                stop=True,
                            )
                        diag = j * 128 >= Q * 512
                        if diag and variant != "v11":
                            for hi in range(2):
                                nc.vector.tensor_add(
                                    ps[:, hi, qoff:qoff + 128],
                                    ps[:, hi, qoff:qoff + 128],
                                    tri[:],
                                )
                        et = ep.tile([128, 2, 512], F32R, tag="exp", name=f"et{j}")
                        nc.scalar.activation(
                            et[:, :, qoff:], ps[:, :, qoff:], AF.Exp, scale=0.125
                        )
                        if diag and variant == "v11":
                            # zero masked entries after exp, off the PE->ACT chain
                            for hi in range(2):
                                nc.vector.tensor_mul(
                                    et[:, hi, qoff:qoff + 128],
                                    et[:, hi, qoff:qoff + 128],
                                    tri01[:],
                                )
                        return et

                    def emit_attn(j, et):
                        tbj, sub = j // 4, j % 4
                        qoff = max(0, j * 128 - Q * 512)
                        for hi, acc in ((0, acc0), (1, acc1)):
                            nc.tensor.matmul(
                                acc[0:65, qoff:],
                                vv[tbj][:, sub, hp * 2 + hi, :],
                                et[:, hi, qoff:],
                                start=(j == 0),
                                stop=(j == nj - 1),
                            )

                    # software pipeline: scores/exp run up to two k-tiles
                    # ahead of the accumulating attn matmuls so the in-order
                    # PE stream never head-blocks on the ACT exp.
                    depth = {"v3": 1, "v6": 3}.get(variant, 2)
                    ets = [emit_scores(j) for j in range(min(depth, nj))]
                    for j in range(depth, nj):
                        ets.append(emit_scores(j))
                        emit_attn(j - depth, ets[j - depth])
                    for j in range(max(0, nj - depth), nj):
                        emit_attn(j, ets[j])
                    if variant in ("v9",):
                        accs_sb = []
                        for hi, acc in ((0, acc0), (1, acc1)):
                            asb = normp.tile([128, 512], F32, tag=f"asb{hi}")
                            nc.vector.tensor_copy(asb[0:65, :], acc[0:65, :])
                            accs_sb.append(asb)
                        for hi, asb in ((0, accs_sb[0]), (1, accs_sb[1])):
                            sr = normp.tile([1, 512], F32, tag="sr")
                            nc.vector.tensor_copy(sr[0:1, :], asb[64:65, :])
                            bb = normp.tile([64, 512], F32, tag="bb")
                            nc.gpsimd.partition_broadcast(bb[:], sr[0:1, :])
                            rb = normp.tile([64, 512], F32, tag="rb")
                            nc.vector.reciprocal(rb[:], bb[:])
                            nc.vector.tensor_mul(
                                attnT[hp][hi * 64:(hi + 1) * 64, :],
                                asb[0:64, :],
                                rb[:],
                            )
                        continue_norm = False
                    else:
                        continue_norm = True
                    for hi, acc in (((0, acc0), (1, acc1)) if continue_norm else ()):
                        if variant == "v3":
                            sr = normp.tile([128, 512], F32R, tag="srr")
                            nc.vector.tensor_copy(sr[64:65, :], acc[64:65, :])
                            pb = psS.tile([128, 512], F32, tag="sc")
                            nc.tensor.matmul(
                                pb[0:64, :], ones_r[64:65, 0:64], sr[64:65, :],
                                start=True, stop=True,
                            )
                            rb = normp.tile([64, 512], F32, tag="rb")
                            nc.vector.reciprocal(rb[:], pb[0:64, :])
                        else:
                            # sum row -> DMA partition-broadcast -> wide
                            # reciprocal -> normalize (no PE/ACT involvement)
                            sr = normp.tile([1, 512], F32, tag="sr")
                            nc.vector.tensor_copy(sr[0:1, :], acc[64:65, :])
                            bb = normp.tile([64, 512], F32, tag="bb")
                            nc.gpsimd.partition_broadcast(bb[:], sr[0:1, :])
                            rb = normp.tile([64, 512], F32, tag="rb")
                            nc.vector.reciprocal(rb[:], bb[:])
                        nc.vector.tensor_mul(
                            attnT[hp][hi * 64:(hi + 1) * 64, :],
                            acc[0:64, :],
                            rb[:],
                        )
                # Wo projection for this token block
                for st_ in range(4):
                    ob = obp.tile([128, D], F32, tag="ob")
                    for nh in range(2):
                        po = psS.tile([128, 512], F32, tag="sc")
                        for kt in range(4):
                            nc.tensor.matmul(
                                po[:],
                                attnT[kt][:, st_ * 128:(st_ + 1) * 128],
                                wo_r[:, kt, nh * 512:(nh + 1) * 512],
                                start=(kt == 0),
                                stop=(kt == 3),
                            )
                        nc.vector.tensor_copy(ob[:, nh * 512:(nh + 1) * 512], po[:])
                    r0 = Q * 512 + st_ * 128
                    nc.sync.dma_start(out_ap[r0:r0 + 128, :], ob[:])

            if mode == "full":
                for rep in range(repeat):
                    if variant == "v10":
                        phase1(0, rep)
                        phase1(1, rep)
                        attention(0, rep)
                        phase1(2, rep)
                        attention(1, rep)
                        phase1(3, rep)
                        attention(2, rep)
                        attention(3, rep)
                    else:
                        for tb in range(TB):
                            phase1(tb, rep)
                        for Q in range(TB):
                            if Q == 0 and variant == "v12":
                                with tc.high_priority():
                                    attention(Q, rep)
                            else:
                                attention(Q, rep)
            elif mode == "p1":
                for rep in range(repeat):
                    for tb in range(TB):
                        phase1(tb, rep)
                for Q in range(TB):
                    attention(Q, 0)
            elif mode == "attn":
                for tb in range(TB):
                    phase1(tb, 0)
                for rep in range(repeat):
                    for Q in range(TB):
                        attention(Q, rep)

    nc.compile()
    return nc


def build13(repeat: int = 1, mode: str = "full", variant: str = "v13"):
    """bf16 rework: every matmul input bf16 (fp32 PSUM accumulation),
    host-cast bf16 weights DMA'd directly, causal mask folded into the
    scores accumulation as an ident^T@tri matmul on PE, v-bias via DVE
    add (no PE bias pass), PSUM retiled to 1-bank ring(4) + 2-bank
    ring(2), bf16 output partials."""
    dmat = variant.endswith("t")
    dvemask = variant[-1] in ("t", "m")
    nc = bacc.Bacc("TRN2", target_bir_lowering=False, debug=False)
    x_aps = {
        n: nc.dram_tensor(n, [S, D], BF16, kind="ExternalInput").ap()
        for n in ("x_q", "x_k", "x_v")
    }
    wq_ap = nc.dram_tensor("wq", [D, DL], BF16, kind="ExternalInput").ap()
    bq_ap = nc.dram_tensor("bq", [DL], F32, kind="ExternalInput").ap()
    wo_ap = nc.dram_tensor("wo", [DL, D], BF16, kind="ExternalInput").ap()
    tri_ap = nc.dram_tensor("tri", [128, 128], BF16, kind="ExternalInput").ap()
    id_ap = nc.dram_tensor("ident", [128, 128], BF16, kind="ExternalInput").ap()
    out_ap = nc.dram_tensor("out", [S, D], BF16, kind="ExternalOutput").ap()

    with tile.TileContext(nc) as tc:
        with tc.tile_pool(name="const", bufs=1) as const, \
             tc.tile_pool(name="persist", bufs=1) as persist, \
             tc.tile_pool(name="xn", bufs=4) as xnp, \
             tc.tile_pool(name="xt", bufs=2) as xtp, \
             tc.tile_pool(name="qt", bufs=4) as qtp, \
             tc.tile_pool(name="exp", bufs=3) as ep, \
             tc.tile_pool(name="at", bufs=2) as atp, \
             tc.tile_pool(name="norm", bufs=2) as normp, \
             tc.tile_pool(name="ob", bufs=2) as obp, \
             tc.tile_pool(name="ps", bufs=2, space="PSUM") as ps:

            # ---- constants (small DMAs off the main SP queue) ----
            ident_b = const.tile([128, 128], BF16)
            nc.scalar.dma_start(ident_b[:], id_ap[:])
            tri_b = const.tile([128, 128], BF16)
            nc.scalar.dma_start(tri_b[:], tri_ap[:])
            bq_sb = const.tile([128, 4], F32)
            nc.scalar.dma_start(bq_sb[:], bq_ap.rearrange("(t p) -> p t", p=128))
            bq_row = const.tile([1, DL], F32)
            nc.scalar.dma_start(bq_row[:], bq_ap.rearrange("(a n) -> a n", a=1))
            bq_bcast = const.tile([128, DL], F32)
            nc.gpsimd.partition_broadcast(bq_bcast[:], bq_row[0:1, :])

            # ---- weights: host-cast bf16, direct DMA (scalar queue) ----
            wq_b = persist.tile([128, JD, DL], BF16)
            for j in range(JD):
                nc.scalar.dma_start(wq_b[:, j, :], wq_ap[j * 128:(j + 1) * 128, :])
            wo_b = persist.tile([128, 4, D], BF16)
            for kt in range(4):
                nc.scalar.dma_start(wo_b[:, kt, :], wo_ap[kt * 128:(kt + 1) * 128, :])

            # persistent per-token-block tensors
            kT = [persist.tile([128, 4, 512], BF16, name=f"kT{i}", tag=f"kT{i}")
                  for i in range(TB)]
            vv = [persist.tile([128, 4, NHL, HD + 1], BF16, name=f"vv{i}",
                               tag=f"vv{i}") for i in range(TB)]
            for i in range(TB):
                nc.vector.memset(vv[i][:, :, :, HD], 1.0)

            q_tiles = [None] * TB

            def phase1(tb, rep):
                """transpose + project q,k,v for token block tb (512 tokens)."""
                for name in ("x_k", "x_v", "x_q"):
                    x_ap = x_aps[name]
                    xT = xtp.tile([128, JD, 512], BF16, tag="xt")
                    if dmat:
                        # hw xbar transpose on the DMA path: no PE transposes,
                        # no PSUM staging, no SBUF copies
                        for j in range(JD):
                            nc.sync.dma_start_transpose(
                                xT[:, j, :],
                                x_ap[tb * 512:(tb + 1) * 512,
                                     j * 128:(j + 1) * 128],
                            )
                    else:
                      for sub in range(4):
                        r0 = tb * 512 + sub * 128
                        xh = []
                        for half in range(2):
                            xn = xnp.tile([128, DL], BF16, tag=f"xn{half}")
                            nc.sync.dma_start(
                                xn[:], x_ap[r0:r0 + 128, half * DL:(half + 1) * DL]
                            )
                            xh.append(xn)
                        pt = ps.tile([128, JD, 128], BF16, tag="acc", bufs=4)
                        for j in range(JD):
                            nc.tensor.transpose(
                                pt[:, j, :],
                                xh[j // 4][:, (j % 4) * 128:(j % 4 + 1) * 128],
                                ident_b[:],
                            )
                        dst = xT[:, :, sub * 128:(sub + 1) * 128]
                        if sub % 2 == 0:
                            nc.vector.tensor_copy(dst, pt[:])
                        else:
                            nc.scalar.activation(dst, pt[:], AF.Identity)
                    if name == "x_v":
                        vt = vv[tb]
                        for sub in range(4):
                            pv = ps.tile([128, 512], F32, tag="acc", bufs=4)
                            for j in range(JD):
                                nc.tensor.matmul(
                                    pv[:],
                                    xT[:, j, sub * 128:(sub + 1) * 128],
                                    wq_b[:, j, :],
                                    start=(j == 0),
                                    stop=(j == JD - 1),
                                )
                            nc.vector.tensor_add(
                                vt[:, sub, :, 0:HD],
                                pv[:].rearrange("p (h d) -> p h d", h=NHL),
                                bq_bcast[:].rearrange("p (h d) -> p h d", h=NHL),
                            )
                    else:
                        if name == "x_q":
                            dest = qtp.tile([128, 4, 512], BF16, tag="qt")
                            q_tiles[tb] = dest
                        else:
                            dest = kT[tb]
                        for dt_ in range(4):
                            py = ps.tile([128, 512], F32, tag="acc", bufs=4)
                            for j in range(JD):
                                nc.tensor.matmul(
                                    py[:],
                                    wq_b[:, j, dt_ * 128:(dt_ + 1) * 128],
                                    xT[:, j, :],
                                    start=(j == 0),
                                    stop=(j == JD - 1),
                                )
                            nc.scalar.activation(
                                dest[:, dt_, :],
                                py[:],
                                AF.Identity,
                                bias=bq_sb[:, dt_:dt_ + 1],
                            )

            def attention(Q, rep):
                """attention + Wo for query block Q (512 tokens)."""
                attnT = [atp.tile([128, 512], BF16, tag=f"at{i}", name=f"attnT{i}")
                         for i in range(4)]
                qtile = q_tiles[Q]
                nj = 4 * (Q + 1)
                for hp in range(4):
                    acc0 = ps.tile([128, 512], F32, tag="acc", bufs=4)
                    acc1 = ps.tile([128, 512], F32, tag="acc", bufs=4)

                    def emit_scores(j):
                        """scoresT pair + fused mask + exp for k-tile j."""
                        tbj, sub = j // 4, j % 4
                        qoff = max(0, j * 128 - Q * 512)
                        diag = j * 128 >= Q * 512
                        ps2 = ps.tile([128, 2, 512], F32, tag="ps2", name=f"ps{j}")
                        for hi, base in ((0, 0), (1, 64)):
                            if diag and dvemask:
                                nc.tensor.matmul(
                                    ps2[:, hi, qoff:],
                                    kT[tbj][base:base + 64, hp,
                                            sub * 128:(sub + 1) * 128],
                                    qtile[base:base + 64, hp, qoff:],
                                    start=True,
                                    stop=True,
                                )
                                nc.vector.tensor_add(
                                    ps2[:, hi, qoff:qoff + 128],
                                    ps2[:, hi, qoff:qoff + 128],
                                    tri_b[:],
                                )
                            elif diag:
                                # diagonal: 128-wide q window gets tri added
                                # via ident^T@tri accumulation on PE
                                nc.tensor.matmul(
                                    ps2[:, hi, qoff:qoff + 128],
                                    kT[tbj][base:base + 64, hp,
                                            sub * 128:(sub + 1) * 128],
                                    qtile[base:base + 64, hp, qoff:qoff + 128],
                                    start=True,
                                    stop=False,
                                )
                                nc.tensor.matmul(
                                    ps2[:, hi, qoff:qoff + 128],
                                    ident_b[:],
                                    tri_b[:],
                                    start=False,
                                    stop=True,
                                )
                                if qoff + 128 < 512:
                                    nc.tensor.matmul(
                                        ps2[:, hi, qoff + 128:],
                                        kT[tbj][base:base + 64, hp,
                                                sub * 128:(sub + 1) * 128],
                                        qtile[base:base + 64, hp, qoff + 128:],
                                        start=True,
                                        stop=True,
                                    )
                            else:
                                nc.tensor.matmul(
                                    ps2[:, hi, :],
                                    kT[tbj][base:base + 64, hp,
                                            sub * 128:(sub + 1) * 128],
                                    qtile[base:base + 64, hp, :],
                                    start=True,
                                    stop=True,
                                )
                        et = ep.tile([128, 2, 512], BF16, tag="exp", name=f"et{j}")
                        nc.scalar.activation(
                            et[:, :, qoff:], ps2[:, :, qoff:], AF.Exp, scale=0.125
                        )
                        return et

                    def emit_attn(j, et):
                        tbj, sub = j // 4, j % 4
                        qoff = max(0, j * 128 - Q * 512)
                        for hi, acc in ((0, acc0), (1, acc1)):
                            nc.tensor.matmul(
                                acc[0:65, qoff:],
                                vv[tbj][:, sub, hp * 2 + hi, :],
                                et[:, hi, qoff:],
                                start=(j == 0),
                                stop=(j == nj - 1),
                            )

                    depth = 2
                    ets = [emit_scores(j) for j in range(min(depth, nj))]
                    for j in range(depth, nj):
                        ets.append(emit_scores(j))
                        emit_attn(j - depth, ets[j - depth])
                    for j in range(max(0, nj - depth), nj):
                        emit_attn(j, ets[j])

                    for hi, acc in ((0, acc0), (1, acc1)):
                        rb1 = normp.tile([1, 512], F32, tag="rb")
                        nc.vector.reciprocal(rb1[0:1, :], acc[64:65, :])
                        bb = normp.tile([64, 512], F32, tag="bb")
                        nc.gpsimd.partition_broadcast(bb[:], rb1[0:1, :])
                        nc.vector.tensor_mul(
                            attnT[hp][hi * 64:(hi + 1) * 64, :],
                            acc[0:64, :],
                            bb[:],
                        )
                # Wo projection for this token block
                for st_ in range(4):
                    ob = obp.tile([128, D], BF16, tag="ob")
                    for nh in range(2):
                        po = ps.tile([128, 512], F32, tag="acc", bufs=4)
                        for kt in range(4):
                            nc.tensor.matmul(
                                po[:],
                                attnT[kt][:, st_ * 128:(st_ + 1) * 128],
                                wo_b[:, kt, nh * 512:(nh + 1) * 512],
                                start=(kt == 0),
                                stop=(kt == 3),
                            )
                        nc.vector.tensor_copy(ob[:, nh * 512:(nh + 1) * 512], po[:])
                    r0 = Q * 512 + st_ * 128
                    nc.sync.dma_start(out_ap[r0:r0 + 128, :], ob[:])

            for rep in range(repeat):
                if mode == "interleave":
                    phase1(0, rep)
                    phase1(1, rep)
                    attention(0, rep)
                    phase1(2, rep)
                    attention(1, rep)
                    phase1(3, rep)
                    attention(2, rep)
                    attention(3, rep)
                else:
                    for tb in range(TB):
                        phase1(tb, rep)
                    for Q in range(TB):
                        attention(Q, rep)

    nc.compile()
    return nc


_BUILD_CACHE = {}


def _get(repeat=1, mode="full", variant="v4"):
    key = (repeat, mode, variant)
    if key not in _BUILD_CACHE:
        if variant.startswith("v13"):
            _BUILD_CACHE[key] = build13(repeat, mode, variant)
        else:
            _BUILD_CACHE[key] = build(repeat, mode, variant)
    return _BUILD_CACHE[key]


def make_in_maps(q, k, v, Wq, bq, Wo, bo, variant="v4"):
    import ml_dtypes
    bf16 = ml_dtypes.bfloat16
    if variant.startswith("v13"):
        tri = np.where(
            np.arange(128)[:, None] <= np.arange(128)[None, :], 0.0, NEG
        ).astype(bf16)
        ident = np.eye(128, dtype=bf16)
        in_maps = []
        for c in range(8):
            b, g = c // 2, c % 2
            sl = slice(g * DL, (g + 1) * DL)
            in_maps.append({
                "x_q": np.ascontiguousarray(q[b]).astype(bf16),
                "x_k": np.ascontiguousarray(k[b]).astype(bf16),
                "x_v": np.ascontiguousarray(v[b]).astype(bf16),
                "wq": np.ascontiguousarray(Wq[:, sl]).astype(bf16),
                "bq": np.ascontiguousarray(bq[sl]),
                "wo": np.ascontiguousarray(Wo[sl, :]).astype(bf16),
                "tri": tri,
                "ident": ident,
            })
        return in_maps
    xdt = bf16 if variant == "v7" else np.float32
    tri = np.where(
        np.arange(128)[:, None] <= np.arange(128)[None, :], 0.0, NEG
    ).astype(np.float32)
    tri01 = (tri == 0.0).astype(np.float32)
    ident = np.eye(128, dtype=np.float32)
    in_maps = []
    for c in range(8):
        b, g = c // 2, c % 2
        sl = slice(g * DL, (g + 1) * DL)
        in_maps.append({
            "x_q": np.ascontiguousarray(q[b]).astype(xdt),
            "x_k": np.ascontiguousarray(k[b]).astype(xdt),
            "x_v": np.ascontiguousarray(v[b]).astype(xdt),
            "wq": np.ascontiguousarray(Wq[:, sl]),
            "bq": np.ascontiguousarray(bq[sl]),
            "wo": np.ascontiguousarray(Wo[sl, :]),
            "tri": tri,
            "tri01": tri01,
            "ident": ident,
        })
    return in_maps


DEFAULT_VARIANT = "v13"
DEFAULT_MODE = "full"


def kernel(q, k, v, Wq, bq, Wo, bo):
    q, k, v, Wq, bq, Wo, bo = (
        np.asarray(a, dtype=np.float32) for a in (q, k, v, Wq, bq, Wo, bo)
    )
    nc = _get(1, DEFAULT_MODE, DEFAULT_VARIANT)
    in_maps = make_in_maps(q, k, v, Wq, bq, Wo, bo, DEFAULT_VARIANT)
    res = run_bass_kernel_spmd(nc, in_maps, list(range(8)))
    B = q.shape[0]
    out = np.empty((B, S, D), dtype=np.float32)
    for b in range(B):
        out[b] = (res.results[2 * b]["out"].astype(np.float32)
                  + res.results[2 * b + 1]["out"].astype(np.float32) + bo)
    return out



# revision 12
# speedup vs baseline: 1.7973x; 1.1067x over previous
"""Causal MHA (shared q_linear) Bass kernel for 8 TRN2 NeuronCores.

Sharding: core c handles batch b=c//2, head-group g=c%2 (8 of 16 heads,
columns 512g:512g+512 of the shared projection).  Each core computes a
partial output (its head-group's contribution through Wo); the host sums
the two partials per batch and adds bo.

Compute layout (per core, S=2048 tokens, D=1024, 8 heads of hd=64):
  xT  = transpose(x) via PE               [1024, 2048]  (fp32, exact)
  qT/kT = Wq_g^T @ xT (+bq)               [512, 2048]   (fp32r matmuls)
  v   = x @ Wq_g (+bq), stored [tok, head, 65] with a fused ones column
  scoresT[k,q] = kh @ qh^T (per head, K=64, two heads packed in PE rows)
  exp on ACT with scale=1/8, additive -1e10 causal mask on PSUM
  attnT[hd+1, q] = [vh|1]^T @ expT  accumulated over k in PSUM
     row 64 = sum(exp) -> reciprocal -> rank-1 PE broadcast -> normalize
  out = attnT^T @ Wo_g  (partial, host adds the two head-groups + bo)
"""

import sys

sys.path.insert(0, "/opt/trn_rl_repo")

import numpy as np
import concourse.bass as bass  # noqa: F401
import concourse.tile as tile
from concourse import bacc, mybir
from concourse.bass_utils import run_bass_kernel_spmd

F32 = mybir.dt.float32
F32R = mybir.dt.float32r
BF16 = mybir.dt.bfloat16
AF = mybir.ActivationFunctionType

S = 2048          # tokens
D = 1024          # model dim
DL = 512          # local (per-core) projection columns = 8 heads * 64
HD = 64           # head dim
NHL = 8           # local heads
TB = 4            # token blocks of 512
JD = 8            # Din blocks of 128
NEG = -1.0e10


def build(repeat: int = 1, mode: str = "full", variant: str = "v4"):
    nc = bacc.Bacc("TRN2", target_bir_lowering=False, debug=False)
    xdt = BF16 if variant == "v7" else F32
    x_aps = {
        n: nc.dram_tensor(n, [S, D], xdt, kind="ExternalInput").ap()
        for n in ("x_q", "x_k", "x_v")
    }
    wq_ap = nc.dram_tensor("wq", [D, DL], F32, kind="ExternalInput").ap()
    bq_ap = nc.dram_tensor("bq", [DL], F32, kind="ExternalInput").ap()
    wo_ap = nc.dram_tensor("wo", [DL, D], F32, kind="ExternalInput").ap()
    tri_ap = nc.dram_tensor("tri", [128, 128], F32, kind="ExternalInput").ap()
    tri01_ap = nc.dram_tensor("tri01", [128, 128], F32, kind="ExternalInput").ap()
    id_ap = nc.dram_tensor("ident", [128, 128], F32, kind="ExternalInput").ap()
    out_ap = nc.dram_tensor("out", [S, D], F32, kind="ExternalOutput").ap()

    with tile.TileContext(nc) as tc:
        with tc.tile_pool(name="const", bufs=1) as const, \
             tc.tile_pool(name="persist", bufs=1) as persist, \
                          tc.tile_pool(name="xn", bufs=3) as xnp, \
             tc.tile_pool(name="xt", bufs=1) as xtp, \
             tc.tile_pool(name="qt", bufs=4) as qtp, \
             tc.tile_pool(name="exp", bufs=(2 if variant == "v9" else 3)) as ep, \
             tc.tile_pool(name="at", bufs=2) as atp, \
             tc.tile_pool(name="norm", bufs=1) as normp, \
             tc.tile_pool(name="ob", bufs=2) as obp, \
             tc.tile_pool(name="psS", bufs=(2 if variant == "v8" else 3), space="PSUM") as psS, \
             tc.tile_pool(name="psAcc", bufs=(3 if variant == "v8" else 2), space="PSUM") as psAcc:

            # ---- constants ----
            ident = const.tile([128, 128], F32)
            nc.sync.dma_start(ident[:], id_ap[:])
            tri = const.tile([128, 128], F32)
            nc.sync.dma_start(tri[:], tri_ap[:])
            tri01 = const.tile([128, 128], F32)
            nc.sync.dma_start(tri01[:], tri01_ap[:])
            bq_sb = const.tile([128, 4], F32)
            nc.sync.dma_start(bq_sb[:], bq_ap.rearrange("(t p) -> p t", p=128))
            bq_row = const.tile([1, DL], F32)
            nc.sync.dma_start(bq_row[:], bq_ap.rearrange("(a n) -> a n", a=1))
            bq_row_r = const.tile([1, DL], F32R)
            nc.vector.tensor_copy(bq_row_r[:], bq_row[:])
            ones_f = const.tile([128, 128], F32)
            nc.vector.memset(ones_f[:], 1.0)
            ones_r = const.tile([128, 128], F32R)
            nc.vector.tensor_copy(ones_r[:], ones_f[:])
            if variant == "v7":
                ones_b = const.tile([128, 128], BF16)
                nc.vector.tensor_copy(ones_b[:], ones_f[:])
                bq_row_b = const.tile([1, DL], BF16)
                nc.vector.tensor_copy(bq_row_b[:], bq_row[:])

            # ---- weights (cast to fp32r once) ----
            wq_r = persist.tile([128, JD, DL], BF16 if variant == "v7" else F32R)
            for j in range(JD):
                st = obp.tile([128, D], F32, tag="ob")
                nc.sync.dma_start(st[:, 0:DL], wq_ap[j * 128:(j + 1) * 128, :])
                nc.vector.tensor_copy(wq_r[:, j, :], st[:, 0:DL])
            wo_r = persist.tile([128, 4, D], F32R)
            for kt in range(4):
                st = obp.tile([128, D], F32, tag="ob")
                nc.sync.dma_start(st[:], wo_ap[kt * 128:(kt + 1) * 128, :])
                nc.vector.tensor_copy(wo_r[:, kt, :], st[:])

            # persistent per-token-block tensors
            kT = [persist.tile([128, 4, 512], F32R, name=f"kT{i}", tag=f"kT{i}") for i in range(TB)]
            vv = [persist.tile([128, 4, NHL, HD + 1], F32R, name=f"vv{i}", tag=f"vv{i}") for i in range(TB)]

            q_tiles = [None] * TB

            def _phase1_transpose(x_ap, xT, tb):
                for sub in range(4):
                    r0 = tb * 512 + sub * 128
                    # two half-tiles so transposes of D-cols 0:512 start as
                    # soon as the first 256KB lands (whole-tile dep otherwise
                    # stalls PE ~2.7us at every input boundary)
                    xh = []
                    for half in range(2):
                        xn = xnp.tile([128, DL], F32, tag=f"xn{half}")
                        nc.sync.dma_start(
                            xn[:], x_ap[r0:r0 + 128, half * DL:(half + 1) * DL]
                        )
                        xh.append(xn)
                    if variant not in ("v5", "v6"):
                        for jg in range(2):
                            pt = psS.tile([128, 512], F32, tag="sc")
                            for ji in range(4):
                                j = jg * 4 + ji
                                nc.tensor.transpose(
                                    pt[:, ji * 128:(ji + 1) * 128],
                                    xh[jg][:, ji * 128:(ji + 1) * 128],
                                    ident[:],
                                )
                            dst = xT[:, jg * 4:(jg + 1) * 4,
                                     sub * 128:(sub + 1) * 128]
                            srcv = pt[:].rearrange("p (j t) -> p j t", j=4)
                            if jg == 0 or variant == "v3":
                                nc.vector.tensor_copy(dst, srcv)
                            else:
                                nc.scalar.activation(dst, srcv, AF.Identity)
                    else:
                        pt = psS.tile([128, 2, 512], F32, tag="sc")
                        for j in range(JD):
                            nc.tensor.transpose(
                                pt[:, j // 4, (j % 4) * 128:(j % 4 + 1) * 128],
                                xn[:, j * 128:(j + 1) * 128],
                                ident[:],
                            )
                        dst = xT[:, :, sub * 128:(sub + 1) * 128]
                        srcv = pt[:].rearrange("p b (g t) -> p (b g) t", g=4)
                        if sub % 2 == 0:
                            nc.vector.tensor_copy(dst, srcv)
                        else:
                            nc.scalar.activation(dst, srcv, AF.Identity)

            def phase1(tb, rep):
                """transpose + project q,k,v for token block tb (512 tokens)."""
                for name in ("x_k", "x_v", "x_q"):
                    x_ap = x_aps[name]
                    if variant == "v7":
                        xT = xtp.tile([128, JD, 512], BF16, tag="xt")
                        for j in range(JD):
                            nc.scalar.dma_start(
                                out=xT[:, j, :],
                                in_=x_ap[tb * 512:(tb + 1) * 512,
                                         j * 128:(j + 1) * 128],
                                transpose=True,
                            )
                    else:
                        xT = xtp.tile([128, JD, 512], F32R, tag="xt")
                        _phase1_transpose(x_ap, xT, tb)
                    if name == "x_v":
                        vt = vv[tb]
                        for sub in range(4):
                            pv = psS.tile([128, 512], F32, tag="sc")
                            for j in range(JD):
                                nc.tensor.matmul(
                                    pv[:],
                                    xT[:, j, sub * 128:(sub + 1) * 128],
                                    wq_r[:, j, :],
                                    start=(j == 0),
                                    stop=False,
                                )
                            nc.tensor.matmul(
                                pv[:],
                                (ones_b if variant == "v7" else ones_r)[0:1, 0:128],
                                (bq_row_b if variant == "v7" else bq_row_r)[:],
                                start=False,
                                stop=True,
                            )
                            nc.vector.tensor_copy(
                                vt[:, sub, :, 0:HD],
                                pv[:].rearrange("p (h d) -> p h d", h=NHL),
                            )
                        nc.vector.tensor_copy(
                            vt[:, :, :, HD],
                            ones_f[:, 0:32].rearrange("p (s h) -> p s h", s=4),
                        )
                    else:
                        if name == "x_q":
                            dest = qtp.tile([128, 4, 512], F32R, tag="qt")
                            q_tiles[tb] = dest
                        else:
                            dest = kT[tb]
                        for dt_ in range(4):
                            py = psS.tile([128, 512], F32, tag="sc")
                            for j in range(JD):
                                nc.tensor.matmul(
                                    py[:],
                                    wq_r[:, j, dt_ * 128:(dt_ + 1) * 128],
                                    xT[:, j, :],
                                    start=(j == 0),
                                    stop=(j == JD - 1),
                                )
                            nc.scalar.activation(
                                dest[:, dt_, :],
                                py[:],
                                AF.Identity,
                                bias=bq_sb[:, dt_:dt_ + 1],
                            )

            def attention(Q, rep):
                """attention + Wo for query block Q (512 tokens)."""
                attnT = [atp.tile([128, 512], F32R, tag=f"at{i}", name=f"attnT{i}")
                         for i in range(4)]
                qtile = q_tiles[Q]
                nj = 4 * (Q + 1)
                for hp in range(4):
                    acc0 = psAcc.tile([128, 512], F32, tag="acc")
                    acc1 = psAcc.tile([128, 512], F32, tag="acc")

                    def emit_scores(j):
                        """scoresT pair + mask + exp for k-tile j; returns exp tile."""
                        tbj, sub = j // 4, j % 4
                        qoff = max(0, j * 128 - Q * 512)
                        ps = psS.tile([128, 2, 512], F32, tag="sc", name=f"ps{j}")
                        for hi, base in ((0, 0), (1, 64)):
                            nc.tensor.matmul(
                                ps[:, hi, qoff:],
                                kT[tbj][base:base + 64, hp,
                                        sub * 128:(sub + 1) * 128],
                                qtile[base:base + 64, hp, qoff:],
                                start=True,
                                stop=True,
                            )
                        diag = j * 128 >= Q * 512
                        if diag and variant != "v11":
                            for hi in range(2):
                                nc.vector.tensor_add(
                                    ps[:, hi, qoff:qoff + 128],
                                    ps[:, hi, qoff:qoff + 128],
                                    tri[:],
                                )
                        et = ep.tile([128, 2, 512], F32R, tag="exp", name=f"et{j}")
                        nc.scalar.activation(
                            et[:, :, qoff:], ps[:, :, qoff:], AF.Exp, scale=0.125
                        )
                        if diag and variant == "v11":
                            # zero masked entries after exp, off the PE->ACT chain
                            for hi in range(2):
                                nc.vector.tensor_mul(
                                    et[:, hi, qoff:qoff + 128],
                                    et[:, hi, qoff:qoff + 128],
                                    tri01[:],
                                )
                        return et

                    def emit_attn(j, et):
                        tbj, sub = j // 4, j % 4
                        qoff = max(0, j * 128 - Q * 512)
                        for hi, acc in ((0, acc0), (1, acc1)):
                            nc.tensor.matmul(
                                acc[0:65, qoff:],
                                vv[tbj][:, sub, hp * 2 + hi, :],
                                et[:, hi, qoff:],
                                start=(j == 0),
                                stop=(j == nj - 1),
                            )

                    # software pipeline: scores/exp run up to two k-tiles
                    # ahead of the accumulating attn matmuls so the in-order
                    # PE stream never head-blocks on the ACT exp.
                    depth = {"v3": 1, "v6": 3}.get(variant, 2)
                    ets = [emit_scores(j) for j in range(min(depth, nj))]
                    for j in range(depth, nj):
                        ets.append(emit_scores(j))
                        emit_attn(j - depth, ets[j - depth])
                    for j in range(max(0, nj - depth), nj):
                        emit_attn(j, ets[j])
                    if variant in ("v9",):
                        accs_sb = []
                        for hi, acc in ((0, acc0), (1, acc1)):
                            asb = normp.tile([128, 512], F32, tag=f"asb{hi}")
                            nc.vector.tensor_copy(asb[0:65, :], acc[0:65, :])
                            accs_sb.append(asb)
                        for hi, asb in ((0, accs_sb[0]), (1, accs_sb[1])):
                            sr = normp.tile([1, 512], F32, tag="sr")
                            nc.vector.tensor_copy(sr[0:1, :], asb[64:65, :])
                            bb = normp.tile([64, 512], F32, tag="bb")
                            nc.gpsimd.partition_broadcast(bb[:], sr[0:1, :])
                            rb = normp.tile([64, 512], F32, tag="rb")
                            nc.vector.reciprocal(rb[:], bb[:])
                            nc.vector.tensor_mul(
                                attnT[hp][hi * 64:(hi + 1) * 64, :],
                                asb[0:64, :],
                                rb[:],
                            )
                        continue_norm = False
                    else:
                        continue_norm = True
                    for hi, acc in (((0, acc0), (1, acc1)) if continue_norm else ()):
                        if variant == "v3":
                            sr = normp.tile([128, 512], F32R, tag="srr")
                            nc.vector.tensor_copy(sr[64:65, :], acc[64:65, :])
                            pb = psS.tile([128, 512], F32, tag="sc")
                            nc.tensor.matmul(
                                pb[0:64, :], ones_r[64:65, 0:64], sr[64:65, :],
                                start=True, stop=True,
                            )
                            rb = normp.tile([64, 512], F32, tag="rb")
                            nc.vector.reciprocal(rb[:], pb[0:64, :])
                        else:
                            # sum row -> DMA partition-broadcast -> wide
                            # reciprocal -> normalize (no PE/ACT involvement)
                            sr = normp.tile([1, 512], F32, tag="sr")
                            nc.vector.tensor_copy(sr[0:1, :], acc[64:65, :])
                            bb = normp.tile([64, 512], F32, tag="bb")
                            nc.gpsimd.partition_broadcast(bb[:], sr[0:1, :])
                            rb = normp.tile([64, 512], F32, tag="rb")
                            nc.vector.reciprocal(rb[:], bb[:])
                        nc.vector.tensor_mul(
                            attnT[hp][hi * 64:(hi + 1) * 64, :],
                            acc[0:64, :],
                            rb[:],
                        )
                # Wo projection for this token block
                for st_ in range(4):
                    ob = obp.tile([128, D], F32, tag="ob")
                    for nh in range(2):
                        po = psS.tile([128, 512], F32, tag="sc")
                        for kt in range(4):
                            nc.tensor.matmul(
                                po[:],
                                attnT[kt][:, st_ * 128:(st_ + 1) * 128],
                                wo_r[:, kt, nh * 512:(nh + 1) * 512],
                                start=(kt == 0),
                                stop=(kt == 3),
                            )
                        nc.vector.tensor_copy(ob[:, nh * 512:(nh + 1) * 512], po[:])
                    r0 = Q * 512 + st_ * 128
                    nc.sync.dma_start(out_ap[r0:r0 + 128, :], ob[:])

            if mode == "full":
                for rep in range(repeat):
                    if variant == "v10":
                        phase1(0, rep)
                        phase1(1, rep)
                        attention(0, rep)
                        phase1(2, rep)
                        attention(1, rep)
                        phase1(3, rep)
                        attention(2, rep)
                        attention(3, rep)
                    else:
                        for tb in range(TB):
                            phase1(tb, rep)
                        for Q in range(TB):
                            if Q == 0 and variant == "v12":
                                with tc.high_priority():
                                    attention(Q, rep)
                            else:
                                attention(Q, rep)
            elif mode == "p1":
                for rep in range(repeat):
                    for tb in range(TB):
                        phase1(tb, rep)
                for Q in range(TB):
                    attention(Q, 0)
            elif mode == "attn":
                for tb in range(TB):
                    phase1(tb, 0)
                for rep in range(repeat):
                    for Q in range(TB):
                        attention(Q, rep)

    nc.compile()
    return nc


def build13(repeat: int = 1, mode: str = "full", variant: str = "v13"):
    """bf16 rework: every matmul input bf16 (fp32 PSUM accumulation),
    host-cast bf16 weights DMA'd directly, causal mask folded into the
    scores accumulation as an ident^T@tri matmul on PE, v-bias via DVE
    add (no PE bias pass), PSUM retiled to 1-bank ring(4) + 2-bank
    ring(2), bf16 output partials."""
    dmat = variant.endswith("t")
    dvemask = variant[-1] in ("t", "m")
    nc = bacc.Bacc("TRN2", target_bir_lowering=False, debug=False)
    x_aps = {
        n: nc.dram_tensor(n, [S, D], BF16, kind="ExternalInput").ap()
        for n in ("x_q", "x_k", "x_v")
    }
    wq_ap = nc.dram_tensor("wq", [D, DL], BF16, kind="ExternalInput").ap()
    bq_ap = nc.dram_tensor("bq", [DL], F32, kind="ExternalInput").ap()
    wo_ap = nc.dram_tensor("wo", [DL, D], BF16, kind="ExternalInput").ap()
    tri_ap = nc.dram_tensor("tri", [128, 128], BF16, kind="ExternalInput").ap()
    id_ap = nc.dram_tensor("ident", [128, 128], BF16, kind="ExternalInput").ap()
    out_ap = nc.dram_tensor("out", [S, D], BF16, kind="ExternalOutput").ap()

    with tile.TileContext(nc) as tc:
        with tc.tile_pool(name="const", bufs=1) as const, \
             tc.tile_pool(name="persist", bufs=1) as persist, \
             tc.tile_pool(name="xn", bufs=4) as xnp, \
             tc.tile_pool(name="xt", bufs=2) as xtp, \
             tc.tile_pool(name="qt", bufs=4) as qtp, \
             tc.tile_pool(name="exp", bufs=3) as ep, \
             tc.tile_pool(name="at", bufs=2) as atp, \
             tc.tile_pool(name="norm", bufs=2) as normp, \
             tc.tile_pool(name="ob", bufs=2) as obp, \
             tc.tile_pool(name="ps", bufs=2, space="PSUM") as ps:

            # ---- constants (small DMAs off the main SP queue) ----
            ident_b = const.tile([128, 128], BF16)
            nc.scalar.dma_start(ident_b[:], id_ap[:])
            tri_b = const.tile([128, 128], BF16)
            nc.scalar.dma_start(tri_b[:], tri_ap[:])
            bq_sb = const.tile([128, 4], F32)
            nc.scalar.dma_start(bq_sb[:], bq_ap.rearrange("(t p) -> p t", p=128))
            bq_row = const.tile([1, DL], F32)
            nc.scalar.dma_start(bq_row[:], bq_ap.rearrange("(a n) -> a n", a=1))
            bq_bcast = const.tile([128, DL], F32)
            nc.gpsimd.partition_broadcast(bq_bcast[:], bq_row[0:1, :])

            # ---- weights: host-cast bf16, direct DMA (scalar queue) ----
            wq_b = persist.tile([128, JD, DL], BF16)
            for j in range(JD):
                nc.scalar.dma_start(wq_b[:, j, :], wq_ap[j * 128:(j + 1) * 128, :])
            wo_b = persist.tile([128, 4, D], BF16)
            for kt in range(4):
                nc.scalar.dma_start(wo_b[:, kt, :], wo_ap[kt * 128:(kt + 1) * 128, :])

            # persistent per-token-block tensors
            kT = [persist.tile([128, 4, 512], BF16, name=f"kT{i}", tag=f"kT{i}")
                  for i in range(TB)]
            vv = [persist.tile([128, 4, NHL, HD + 1], BF16, name=f"vv{i}",
                               tag=f"vv{i}") for i in range(TB)]
            for i in range(TB):
                nc.vector.memset(vv[i][:, :, :, HD], 1.0)

            q_tiles = [None] * TB

            def phase1(tb, rep):
                """transpose + project q,k,v for token block tb (512 tokens)."""
                for name in ("x_k", "x_v", "x_q"):
                    x_ap = x_aps[name]
                    xT = xtp.tile([128, JD, 512], BF16, tag="xt")
                    if dmat:
                        # hw xbar transpose on the DMA path: no PE transposes,
                        # no PSUM staging, no SBUF copies
                        for j in range(JD):
                            nc.sync.dma_start_transpose(
                                xT[:, j, :],
                                x_ap[tb * 512:(tb + 1) * 512,
                                     j * 128:(j + 1) * 128],
                            )
                    else:
                      for sub in range(4):
                        r0 = tb * 512 + sub * 128
                        xh = []
                        for half in range(2):
                            xn = xnp.tile([128, DL], BF16, tag=f"xn{half}")
                            nc.sync.dma_start(
                                xn[:], x_ap[r0:r0 + 128, half * DL:(half + 1) * DL]
                            )
                            xh.append(xn)
                        pt = ps.tile([128, JD, 128], BF16, tag="acc", bufs=4)
                        for j in range(JD):
                            nc.tensor.transpose(
                                pt[:, j, :],
                                xh[j // 4][:, (j % 4) * 128:(j % 4 + 1) * 128],
                                ident_b[:],
                            )
                        dst = xT[:, :, sub * 128:(sub + 1) * 128]
                        if sub % 2 == 0:
                            nc.vector.tensor_copy(dst, pt[:])
                        else:
                            nc.scalar.activation(dst, pt[:], AF.Identity)
                    if name == "x_v":
                        vt = vv[tb]
                        for sub in range(4):
                            pv = ps.tile([128, 512], F32, tag="acc", bufs=4)
                            for j in range(JD):
                                nc.tensor.matmul(
                                    pv[:],
                                    xT[:, j, sub * 128:(sub + 1) * 128],
                                    wq_b[:, j, :],
                                    start=(j == 0),
                                    stop=(j == JD - 1),
                                )
                            nc.vector.tensor_add(
                                vt[:, sub, :, 0:HD],
                                pv[:].rearrange("p (h d) -> p h d", h=NHL),
                                bq_bcast[:].rearrange("p (h d) -> p h d", h=NHL),
                            )
                    else:
                        if name == "x_q":
                            dest = qtp.tile([128, 4, 512], BF16, tag="qt")
                            q_tiles[tb] = dest
                        else:
                            dest = kT[tb]
                        for dt_ in range(4):
                            py = ps.tile([128, 512], F32, tag="acc", bufs=4)
                            for j in range(JD):
                                nc.tensor.matmul(
                                    py[:],
                                    wq_b[:, j, dt_ * 128:(dt_ + 1) * 128],
                                    xT[:, j, :],
                                    start=(j == 0),
                                    stop=(j == JD - 1),
                                )
                            nc.scalar.activation(
                                dest[:, dt_, :],
                                py[:],
                                AF.Identity,
                                bias=bq_sb[:, dt_:dt_ + 1],
                            )

            def attention(Q, rep):
                """attention + Wo for query block Q (512 tokens)."""
                attnT = [atp.tile([128, 512], BF16, tag=f"at{i}", name=f"attnT{i}")
                         for i in range(4)]
                qtile = q_tiles[Q]
                nj = 4 * (Q + 1)
                for hp in range(4):
                    acc0 = ps.tile([128, 512], F32, tag="acc", bufs=4)
                    acc1 = ps.tile([128, 512], F32, tag="acc", bufs=4)

                    def emit_scores(j):
                        """scoresT pair + fused mask + exp for k-tile j."""
                        tbj, sub = j // 4, j % 4
                        qoff = max(0, j * 128 - Q * 512)
                        diag = j * 128 >= Q * 512
                        ps2 = ps.tile([128, 2, 512], F32, tag="ps2", name=f"ps{j}")
                        for hi, base in ((0, 0), (1, 64)):
                            if diag and dvemask:
                                nc.tensor.matmul(
                                    ps2[:, hi, qoff:],
                                    kT[tbj][base:base + 64, hp,
                                            sub * 128:(sub + 1) * 128],
                                    qtile[base:base + 64, hp, qoff:],
                                    start=True,
                                    stop=True,
                                )
                                nc.vector.tensor_add(
                                    ps2[:, hi, qoff:qoff + 128],
                                    ps2[:, hi, qoff:qoff + 128],
                                    tri_b[:],
                                )
                            elif diag:
                                # diagonal: 128-wide q window gets tri added
                                # via ident^T@tri accumulation on PE
                                nc.tensor.matmul(
                                    ps2[:, hi, qoff:qoff + 128],
                                    kT[tbj][base:base + 64, hp,
                                            sub * 128:(sub + 1) * 128],
                                    qtile[base:base + 64, hp, qoff:qoff + 128],
                                    start=True,
                                    stop=False,
                                )
                                nc.tensor.matmul(
                                    ps2[:, hi, qoff:qoff + 128],
                                    ident_b[:],
                                    tri_b[:],
                                    start=False,
                                    stop=True,
                                )
                                if qoff + 128 < 512:
                                    nc.tensor.matmul(
                                        ps2[:, hi, qoff + 128:],
                                        kT[tbj][base:base + 64, hp,
                                                sub * 128:(sub + 1) * 128],
                                        qtile[base:base + 64, hp, qoff + 128:],
                                        start=True,
                                        stop=True,
                                    )
                            else:
                                nc.tensor.matmul(
                                    ps2[:, hi, :],
                                    kT[tbj][base:base + 64, hp,
                                            sub * 128:(sub + 1) * 128],
                                    qtile[base:base + 64, hp, :],
                                    start=True,
                                    stop=True,
                                )
                        et = ep.tile([128, 2, 512], BF16, tag="exp", name=f"et{j}")
                        nc.scalar.activation(
                            et[:, :, qoff:], ps2[:, :, qoff:], AF.Exp, scale=0.125
                        )
                        return et

                    def emit_attn(j, et):
                        tbj, sub = j // 4, j % 4
                        qoff = max(0, j * 128 - Q * 512)
                        for hi, acc in ((0, acc0), (1, acc1)):
                            nc.tensor.matmul(
                                acc[0:65, qoff:],
                                vv[tbj][:, sub, hp * 2 + hi, :],
                                et[:, hi, qoff:],
                                start=(j == 0),
                                stop=(j == nj - 1),
                            )

                    depth = 2
                    ets = [emit_scores(j) for j in range(min(depth, nj))]
                    for j in range(depth, nj):
                        ets.append(emit_scores(j))
                        emit_attn(j - depth, ets[j - depth])
                    for j in range(max(0, nj - depth), nj):
                        emit_attn(j, ets[j])

                    for hi, acc in ((0, acc0), (1, acc1)):
                        rb1 = normp.tile([1, 512], F32, tag="rb")
                        nc.vector.reciprocal(rb1[0:1, :], acc[64:65, :])
                        bb = normp.tile([64, 512], F32, tag="bb")
                        nc.gpsimd.partition_broadcast(bb[:], rb1[0:1, :])
                        nc.vector.tensor_mul(
                            attnT[hp][hi * 64:(hi + 1) * 64, :],
                            acc[0:64, :],
                            bb[:],
                        )
                # Wo projection for this token block
                for st_ in range(4):
                    ob = obp.tile([128, D], BF16, tag="ob")
                    for nh in range(2):
                        po = ps.tile([128, 512], F32, tag="acc", bufs=4)
                        for kt in range(4):
                            nc.tensor.matmul(
                                po[:],
                                attnT[kt][:, st_ * 128:(st_ + 1) * 128],
                                wo_b[:, kt, nh * 512:(nh + 1) * 512],
                                start=(kt == 0),
                                stop=(kt == 3),
                            )
                        nc.vector.tensor_copy(ob[:, nh * 512:(nh + 1) * 512], po[:])
                    r0 = Q * 512 + st_ * 128
                    nc.sync.dma_start(out_ap[r0:r0 + 128, :], ob[:])

            for rep in range(repeat):
                if mode == "interleave":
                    phase1(0, rep)
                    phase1(1, rep)
                    attention(0, rep)
                    phase1(2, rep)
                    attention(1, rep)
                    phase1(3, rep)
                    attention(2, rep)
                    attention(3, rep)
                else:
                    for tb in range(TB):
                        phase1(tb, rep)
                    for Q in range(TB):
                        attention(Q, rep)

    nc.compile()
    return nc


def build14(repeat: int = 1, mode: str = "full", variant: str = "v14"):
    """v13 + dispatch-count reduction: one input DMA per (tb, tensor),
    q/k projections and Wo at N=1024 over tb-pairs, diagonal mask via a
    zero-padded [128,512] tri so diag scores are 2 matmuls not 3."""
    dmat = variant.endswith("t")
    deep = "b" in variant[3:]
    flat = "f" in variant[3:]
    DEPTH = 3 if deep else 2
    PS2_BUFS = 3 if deep else 2
    ACC_BUFS = 2 if deep else 4
    EP_BUFS = 4 if deep else 3
    if flat:
        DEPTH = 3 if deep else 2
        PS2_BUFS = 2
        ACC_BUFS = 4
        EP_BUFS = 5 if deep else 4
    nc = bacc.Bacc("TRN2", target_bir_lowering=False, debug=False)
    x_aps = {
        n: nc.dram_tensor(n, [S, D], BF16, kind="ExternalInput").ap()
        for n in ("x_q", "x_k", "x_v")
    }
    wq_ap = nc.dram_tensor("wq", [D, DL], BF16, kind="ExternalInput").ap()
    bq_ap = nc.dram_tensor("bq", [DL], F32, kind="ExternalInput").ap()
    wo_ap = nc.dram_tensor("wo", [DL, D], BF16, kind="ExternalInput").ap()
    tri_ap = nc.dram_tensor("tri", [128, 512], BF16, kind="ExternalInput").ap()
    id_ap = nc.dram_tensor("ident", [128, 128], BF16, kind="ExternalInput").ap()
    out_ap = nc.dram_tensor("out", [S, D], BF16, kind="ExternalOutput").ap()

    with tile.TileContext(nc) as tc:
        with tc.tile_pool(name="const", bufs=1) as const, \
             tc.tile_pool(name="persist", bufs=1) as persist, \
             tc.tile_pool(name="xn", bufs=3) as xnp, \
             tc.tile_pool(name="xt", bufs=2) as xtp, \
             tc.tile_pool(name="qt", bufs=2) as qtp, \
             tc.tile_pool(name="exp", bufs=EP_BUFS) as ep, \
             tc.tile_pool(name="at", bufs=2) as atp, \
             tc.tile_pool(name="norm", bufs=2) as normp, \
             tc.tile_pool(name="ob", bufs=2) as obp, \
             tc.tile_pool(name="ps", bufs=PS2_BUFS, space="PSUM") as ps:

            ident_b = const.tile([128, 128], BF16)
            nc.scalar.dma_start(ident_b[:], id_ap[:])
            tri_b = const.tile([128, 512], BF16)
            nc.scalar.dma_start(tri_b[:], tri_ap[:])
            bq_sb = const.tile([128, 4], F32)
            nc.scalar.dma_start(bq_sb[:], bq_ap.rearrange("(t p) -> p t", p=128))
            bq_row = const.tile([1, DL], F32)
            nc.scalar.dma_start(bq_row[:], bq_ap.rearrange("(a n) -> a n", a=1))
            bq_bcast = const.tile([128, DL], F32)
            nc.gpsimd.partition_broadcast(bq_bcast[:], bq_row[0:1, :])

            wq_b = persist.tile([128, JD, DL], BF16)
            for j in range(JD):
                nc.scalar.dma_start(wq_b[:, j, :], wq_ap[j * 128:(j + 1) * 128, :])
            wo_b = persist.tile([128, 4, D], BF16)
            for kt in range(4):
                nc.scalar.dma_start(wo_b[:, kt, :], wo_ap[kt * 128:(kt + 1) * 128, :])

            # persistent: kT/q over 1024-token pairs, vv over 512 blocks
            kT2 = [persist.tile([128, 4, 1024], BF16, name=f"kT2{i}", tag=f"kT2{i}")
                   for i in range(2)]
            vv = [persist.tile([128, 4, NHL, HD + 1], BF16, name=f"vv{i}",
                               tag=f"vv{i}") for i in range(TB)]
            for i in range(TB):
                nc.vector.memset(vv[i][:, :, :, HD], 1.0)

            q_tiles2 = [None, None]

            def phase1(p, rep):
                """transpose + project q,k,v for token pair-block p (1024)."""
                for name in ("x_k", "x_v", "x_q"):
                    x_ap = x_aps[name]
                    xT = xtp.tile([128, JD, 1024], BF16, tag="xt")
                    if dmat:
                        for j in range(JD):
                            nc.sync.dma_start_transpose(
                                xT[:, j, :],
                                x_ap[p * 1024:(p + 1) * 1024,
                                     j * 128:(j + 1) * 128],
                            )
                    else:
                        for half in range(2):
                            tb = 2 * p + half
                            xn = xnp.tile([128, 4, D], BF16, tag="xn")
                            nc.sync.dma_start(
                                xn[:],
                                x_ap[tb * 512:(tb + 1) * 512, :].rearrange(
                                    "(s p) d -> p s d", p=128
                                ),
                            )
                            for sub4 in range(4):
                                sub = half * 4 + sub4
                                pt = ps.tile([128, JD, 128], BF16, tag="acc",
                                             bufs=ACC_BUFS)
                                for j in range(JD):
                                    nc.tensor.transpose(
                                        pt[:, j, :],
                                        xn[:, sub4, j * 128:(j + 1) * 128],
                                        ident_b[:],
                                    )
                                dst = xT[:, :, sub * 128:(sub + 1) * 128]
                                if sub % 2 == 0:
                                    nc.vector.tensor_copy(dst, pt[:])
                                else:
                                    nc.scalar.activation(dst, pt[:], AF.Identity)
                    if name == "x_v":
                        for sub in range(8):
                            vt = vv[2 * p + sub // 4]
                            pv = ps.tile([128, 512], F32, tag="acc", bufs=ACC_BUFS)
                            for j in range(JD):
                                nc.tensor.matmul(
                                    pv[:],
                                    xT[:, j, sub * 128:(sub + 1) * 128],
                                    wq_b[:, j, :],
                                    start=(j == 0),
                                    stop=(j == JD - 1),
                                )
                            nc.vector.tensor_add(
                                vt[:, sub % 4, :, 0:HD],
                                pv[:].rearrange("p (h d) -> p h d", h=NHL),
                                bq_bcast[:].rearrange("p (h d) -> p h d", h=NHL),
                            )
                    else:
                        if name == "x_q":
                            dest = qtp.tile([128, 4, 1024], BF16, tag="qt")
                            q_tiles2[p] = dest
                        else:
                            dest = kT2[p]
                        for dt_ in range(4):
                            py = ps.tile([128, 2, 512], F32, tag="ps2")
                            for nh in range(2):
                                for j in range(JD):
                                    nc.tensor.matmul(
                                        py[:, nh, :],
                                        wq_b[:, j, dt_ * 128:(dt_ + 1) * 128],
                                        xT[:, j, nh * 512:(nh + 1) * 512],
                                        start=(j == 0),
                                        stop=(j == JD - 1),
                                    )
                            nc.scalar.activation(
                                dest[:, dt_, :],
                                py[:].rearrange("p a b -> p (a b)"),
                                AF.Identity,
                                bias=bq_sb[:, dt_:dt_ + 1],
                            )

            def attention(Q, rep):
                """attention + Wo for query block Q (512 tokens)."""
                attnT = [atp.tile([128, 512], BF16, tag=f"at{i}", name=f"attnT{i}")
                         for i in range(4)]
                qt2 = q_tiles2[Q // 2]
                qbase = (Q % 2) * 512
                nj = 4 * (Q + 1)
                for hp in range(4):
                    acc0 = ps.tile([128, 512], F32, tag="acc", bufs=ACC_BUFS)
                    acc1 = ps.tile([128, 512], F32, tag="acc", bufs=ACC_BUFS)

                    def emit_scores(j):
                        tbj2, sub8 = j // 8, j % 8
                        qoff = max(0, j * 128 - Q * 512)
                        diag = j * 128 >= Q * 512
                        ps2 = ps.tile([128, 2, 512], F32, tag="ps2", name=f"ps{j}")
                        for hi, base in ((0, 0), (1, 64)):
                            nc.tensor.matmul(
                                ps2[:, hi, qoff:],
                                kT2[tbj2][base:base + 64, hp,
                                          sub8 * 128:(sub8 + 1) * 128],
                                qt2[base:base + 64, hp,
                                    qbase + qoff:qbase + 512],
                                start=True,
                                stop=not diag,
                            )
                            if diag:
                                # zero-padded tri: adds the causal -1e10 on
                                # cols [qoff:qoff+128], zeros beyond
                                nc.tensor.matmul(
                                    ps2[:, hi, qoff:],
                                    ident_b[:],
                                    tri_b[:, 0:512 - qoff],
                                    start=False,
                                    stop=True,
                                )
                        et = ep.tile([128, 2, 512], BF16, tag="exp", name=f"et{j}")
                        nc.scalar.activation(
                            et[:, :, qoff:], ps2[:, :, qoff:], AF.Exp, scale=0.125
                        )
                        return et

                    def emit_attn(j, et):
                        tbj, sub = j // 4, j % 4
                        qoff = max(0, j * 128 - Q * 512)
                        for hi, acc in ((0, acc0), (1, acc1)):
                            nc.tensor.matmul(
                                acc[0:65, qoff:],
                                vv[tbj][:, sub, hp * 2 + hi, :],
                                et[:, hi, qoff:],
                                start=(j == 0),
                                stop=(j == nj - 1),
                            )

                    depth = DEPTH
                    ets = [emit_scores(j) for j in range(min(depth, nj))]
                    for j in range(depth, nj):
                        ets.append(emit_scores(j))
                        emit_attn(j - depth, ets[j - depth])
                    for j in range(max(0, nj - depth), nj):
                        emit_attn(j, ets[j])

                    for hi, acc in ((0, acc0), (1, acc1)):
                        rb1 = normp.tile([1, 512], F32, tag="rb")
                        nc.vector.reciprocal(rb1[0:1, :], acc[64:65, :])
                        bb = normp.tile([64, 512], F32, tag="bb")
                        nc.gpsimd.partition_broadcast(bb[:], rb1[0:1, :])
                        nc.vector.tensor_mul(
                            attnT[hp][hi * 64:(hi + 1) * 64, :],
                            acc[0:64, :],
                            bb[:],
                        )
                # Wo at N=1024: 4 matmuls per token sub-block
                for st_ in range(4):
                    ob = obp.tile([128, D], BF16, tag="ob")
                    po = ps.tile([128, 2, 512], F32, tag="ps2")
                    for nh in range(2):
                        for kt in range(4):
                            nc.tensor.matmul(
                                po[:, nh, :],
                                attnT[kt][:, st_ * 128:(st_ + 1) * 128],
                                wo_b[:, kt, nh * 512:(nh + 1) * 512],
                                start=(kt == 0),
                                stop=(kt == 3),
                            )
                    nc.vector.tensor_copy(ob[:], po[:].rearrange("p a b -> p (a b)"))
                    r0 = Q * 512 + st_ * 128
                    nc.sync.dma_start(out_ap[r0:r0 + 128, :], ob[:])

            def emit_wo(attnT, Qp):
                for st_ in range(4):
                    ob = obp.tile([128, D], BF16, tag="ob")
                    po = ps.tile([128, 2, 512], F32, tag="ps2")
                    for nh in range(2):
                        for kt in range(4):
                            nc.tensor.matmul(
                                po[:, nh, :],
                                attnT[kt][:, st_ * 128:(st_ + 1) * 128],
                                wo_b[:, kt, nh * 512:(nh + 1) * 512],
                                start=(kt == 0),
                                stop=(kt == 3),
                            )
                    nc.vector.tensor_copy(ob[:], po[:].rearrange("p a b -> p (a b)"))
                    r0 = Qp * 512 + st_ * 128
                    nc.sync.dma_start(out_ap[r0:r0 + 128, :], ob[:])

            def attention_flat(Q, rep, pending):
                """single software pipeline across all (hp, j) of this Q;
                the previous Q's Wo is emitted into the warmup gap."""
                attnT = [atp.tile([128, 512], BF16, tag=f"at{i}",
                                  name=f"attnT{i}") for i in range(4)]
                qt2 = q_tiles2[Q // 2]
                qbase = (Q % 2) * 512
                nj = 4 * (Q + 1)
                accs = [None] * 4

                def emit_scores_f(hp, j):
                    tbj2, sub8 = j // 8, j % 8
                    qoff = max(0, j * 128 - Q * 512)
                    diag = j * 128 >= Q * 512
                    ps2 = ps.tile([128, 2, 512], F32, tag="ps2", name=f"ps{j}")
                    for hi, base in ((0, 0), (1, 64)):
                        nc.tensor.matmul(
                            ps2[:, hi, qoff:],
                            kT2[tbj2][base:base + 64, hp,
                                      sub8 * 128:(sub8 + 1) * 128],
                            qt2[base:base + 64, hp, qbase + qoff:qbase + 512],
                            start=True,
                            stop=True,
                        )
                        if diag:
                            # in-place 128-wide causal add; group check
                            # skipped (accumulate into a closed region)
                            nc.tensor.matmul(
                                ps2[:, hi, qoff:qoff + 128],
                                ident_b[:],
                                tri_b[:, 0:128],
                                start=False,
                                stop=True,
                                skip_group_check=True,
                            )
                    et = ep.tile([128, 2, 512], BF16, tag="exp", name=f"et{j}")
                    nc.scalar.activation(
                        et[:, :, qoff:], ps2[:, :, qoff:], AF.Exp, scale=0.125
                    )
                    return et

                def emit_attn_f(hp, j, et):
                    tbj, sub = j // 4, j % 4
                    qoff = max(0, j * 128 - Q * 512)
                    acc0, acc1 = accs[hp]
                    for hi, acc in ((0, acc0), (1, acc1)):
                        nc.tensor.matmul(
                            acc[0:65, qoff:],
                            vv[tbj][:, sub, hp * 2 + hi, :],
                            et[:, hi, qoff:],
                            start=(j == 0),
                            stop=(j == nj - 1),
                        )
                    if j == nj - 1:
                        for hi, acc in ((0, acc0), (1, acc1)):
                            rb1 = normp.tile([1, 512], F32, tag="rb")
                            nc.vector.reciprocal(rb1[0:1, :], acc[64:65, :])
                            bb = normp.tile([64, 512], F32, tag="bb")
                            nc.gpsimd.partition_broadcast(bb[:], rb1[0:1, :])
                            nc.vector.tensor_mul(
                                attnT[hp][hi * 64:(hi + 1) * 64, :],
                                acc[0:64, :],
                                bb[:],
                            )

                seq = [(hp, j) for hp in range(4) for j in range(nj)]
                buf = []
                wo_at = min(DEPTH, len(seq) - 1)
                for idx, (hp, j) in enumerate(seq):
                    if j == 0:
                        accs[hp] = (
                            ps.tile([128, 512], F32, tag="acc", bufs=ACC_BUFS,
                                    name=f"acc0h{hp}"),
                            ps.tile([128, 512], F32, tag="acc", bufs=ACC_BUFS,
                                    name=f"acc1h{hp}"),
                        )
                    buf.append((hp, j, emit_scores_f(hp, j)))
                    if idx == wo_at and pending is not None:
                        emit_wo(*pending)
                    if idx >= DEPTH:
                        emit_attn_f(*buf[idx - DEPTH])
                for t in buf[len(seq) - DEPTH:]:
                    emit_attn_f(*t)
                if pending is not None and len(seq) <= wo_at:
                    emit_wo(*pending)
                return attnT

            for rep in range(repeat):
                if mode == "interleave":
                    phase1(0, rep)
                    attention(0, rep)
                    attention(1, rep)
                    phase1(1, rep)
                    attention(2, rep)
                    attention(3, rep)
                elif flat:
                    for p in range(2):
                        phase1(p, rep)
                    pending = None
                    for Q in range(TB):
                        at = attention_flat(Q, rep, pending)
                        pending = (at, Q)
                    emit_wo(*pending)
                else:
                    for p in range(2):
                        phase1(p, rep)
                    for Q in range(TB):
                        attention(Q, rep)

    nc.compile()
    return nc


_BUILD_CACHE = {}


def _get(repeat=1, mode="full", variant="v4"):
    key = (repeat, mode, variant)
    if key not in _BUILD_CACHE:
        if variant.startswith("v14"):
            _BUILD_CACHE[key] = build14(repeat, mode, variant)
        elif variant.startswith("v13"):
            _BUILD_CACHE[key] = build13(repeat, mode, variant)
        else:
            _BUILD_CACHE[key] = build(repeat, mode, variant)
    return _BUILD_CACHE[key]


def make_in_maps(q, k, v, Wq, bq, Wo, bo, variant="v4"):
    import ml_dtypes
    bf16 = ml_dtypes.bfloat16
    if variant.startswith("v14"):
        tri = np.zeros((128, 512), dtype=np.float32)
        tri[:, :128] = np.where(
            np.arange(128)[:, None] <= np.arange(128)[None, :], 0.0, NEG
        )
        tri = tri.astype(bf16)
        ident = np.eye(128, dtype=bf16)
        in_maps = []
        for c in range(8):
            b, g = c // 2, c % 2
            sl = slice(g * DL, (g + 1) * DL)
            in_maps.append({
                "x_q": np.ascontiguousarray(q[b]).astype(bf16),
                "x_k": np.ascontiguousarray(k[b]).astype(bf16),
                "x_v": np.ascontiguousarray(v[b]).astype(bf16),
                "wq": np.ascontiguousarray(Wq[:, sl]).astype(bf16),
                "bq": np.ascontiguousarray(bq[sl]),
                "wo": np.ascontiguousarray(Wo[sl, :]).astype(bf16),
                "tri": tri,
                "ident": ident,
            })
        return in_maps
    if variant.startswith("v13"):
        tri = np.where(
            np.arange(128)[:, None] <= np.arange(128)[None, :], 0.0, NEG
        ).astype(bf16)
        ident = np.eye(128, dtype=bf16)
        in_maps = []
        for c in range(8):
            b, g = c // 2, c % 2
            sl = slice(g * DL, (g + 1) * DL)
            in_maps.append({
                "x_q": np.ascontiguousarray(q[b]).astype(bf16),
                "x_k": np.ascontiguousarray(k[b]).astype(bf16),
                "x_v": np.ascontiguousarray(v[b]).astype(bf16),
                "wq": np.ascontiguousarray(Wq[:, sl]).astype(bf16),
                "bq": np.ascontiguousarray(bq[sl]),
                "wo": np.ascontiguousarray(Wo[sl, :]).astype(bf16),
                "tri": tri,
                "ident": ident,
            })
        return in_maps
    xdt = bf16 if variant == "v7" else np.float32
    tri = np.where(
        np.arange(128)[:, None] <= np.arange(128)[None, :], 0.0, NEG
    ).astype(np.float32)
    tri01 = (tri == 0.0).astype(np.float32)
    ident = np.eye(128, dtype=np.float32)
    in_maps = []
    for c in range(8):
        b, g = c // 2, c % 2
        sl = slice(g * DL, (g + 1) * DL)
        in_maps.append({
            "x_q": np.ascontiguousarray(q[b]).astype(xdt),
            "x_k": np.ascontiguousarray(k[b]).astype(xdt),
            "x_v": np.ascontiguousarray(v[b]).astype(xdt),
            "wq": np.ascontiguousarray(Wq[:, sl]),
            "bq": np.ascontiguousarray(bq[sl]),
            "wo": np.ascontiguousarray(Wo[sl, :]),
            "tri": tri,
            "tri01": tri01,
            "ident": ident,
        })
    return in_maps


DEFAULT_VARIANT = "v14f"
DEFAULT_MODE = "full"


def kernel(q, k, v, Wq, bq, Wo, bo):
    q, k, v, Wq, bq, Wo, bo = (
        np.asarray(a, dtype=np.float32) for a in (q, k, v, Wq, bq, Wo, bo)
    )
    nc = _get(1, DEFAULT_MODE, DEFAULT_VARIANT)
    in_maps = make_in_maps(q, k, v, Wq, bq, Wo, bo, DEFAULT_VARIANT)
    res = run_bass_kernel_spmd(nc, in_maps, list(range(8)))
    B = q.shape[0]
    out = np.empty((B, S, D), dtype=np.float32)
    for b in range(B):
        out[b] = (res.results[2 * b]["out"].astype(np.float32)
                  + res.results[2 * b + 1]["out"].astype(np.float32) + bo)
    return out

